# revision 1
# baseline (speedup 1.0000x reference)
"""GAT layer (segment-softmax message passing) on 8 Trainium2 NeuronCores.

Strategy (per core c of NC=8, SPMD single program, per-core input maps):
  - Nodes sharded by destination: core c owns dst rows [c*NPC, (c+1)*NPC).
  - hT is pre-rolled per core so own nodes are rows [0, NPC). Each core
    computes the full z = h @ W^T itself (no collectives):
      z_all : partition-major pseudo-row layout fp32 [100096, 64] (src gather)
      z_own : row-major fp32 [12544+128, 64]; last 128 rows zeroed (pad target)
  - Edges are grouped by (superbatch of SBB dst-blocks, src chunk window,
    block); each (sb, chunk, block) segment padded to a 128 multiple with
    budgets shared across cores (SPMD geometry). Pad edges gather z_own's
    zero row for dst (=> e=0, ex=1) and carry slot=-1 (one-hot row = 0).
  - Per tile of 128 edges (all one dst block): one-hot ind[e, s] =
    (slot_e == s) built by DVE is_equal against an iota constant; the PE
    accumulates agg[s, 0:65] += ind^T @ [zsrc*ex, ex] into the block's PSUM
    region (start/stop flags per block). No scatter-add, no dedup planning.
  - e = dot(z_src, z_dst) on DVE (fp32); ex = max(exp(e), exp(0.2*e)) on ACT
    (exact leaky-relu+exp identity; softmax shift invariance makes the
    max-subtraction unnecessary at fp32 range for this input).
  - agg lives entirely in SBUF [128, NBLK, 65]; final phase normalizes by
    col 64 (+1e-30) and applies elu, then one strided DMA writes out rows.
"""

import os
import sys

sys.path.insert(0, "/opt/trn_rl_repo")

import numpy as np
import ml_dtypes

import concourse.bacc as bacc
import concourse.mybir as mybir
import concourse.tile as tile
from concourse.bass_utils import run_bass_kernel_spmd

F32 = mybir.dt.float32
BF16 = mybir.dt.bfloat16
I16 = mybir.dt.int16
FP16 = mybir.dt.float16
AF = mybir.ActivationFunctionType
ALU = mybir.AluOpType

LAST_RESULTS = None  # test harness reads exec_time_ns from here
LAST_BUILD = None  # (nc, in_maps, meta) for sim/bench harnesses

N = 100000
E_TOT = 1600000
IN_DIM = 128
OUT_DIM = 64
NC = 8
NPC = N // NC  # 12500
NT_G = (N + 127) // 128  # 782 GEMM row tiles
NROWS = NT_G * 128  # 100096
N_CHUNK = 4
CH_PARTS = 128 // N_CHUNK  # 32
CHUNK_PSEUDO = CH_PARTS * NT_G  # 25024 (< 32768 int16 window)
BLK = 128
NBLK = (NPC + BLK - 1) // BLK  # 98
SBB = 3  # dst blocks per superbatch
NSB = (NBLK + SBB - 1) // SBB  # 33
VD = OUT_DIM + 1  # 65: agg row = [sum ex*z, sum ex]
ZROW = NBLK * BLK  # 12544: index of the zero row in z_own
ZOWN_ROWS = ZROW + 128
GMAX = 8192  # SWDGE per-instruction descriptor cap


def _wrap_idx(idx, budget):
    """[n] int -> [128, budget//16] int16 wrapped + replicated (q7 layout)."""
    a = np.zeros(budget, np.int16)
    a[: len(idx)] = idx.astype(np.int16)
    w = a.reshape(budget // 16, 16).T.copy()  # element i at [i%16, i//16]
    return np.tile(w, (8, 1))


def _plan(src, dst):
    """Shared tile geometry + per-core padded index/slot arrays.

    Returns (geom, per_core) where geom has the shared segment budgets and
    per-tile block/flag metadata, and per_core[c] has gsrc/gdst/slots arrays.
    """
    # seg key = (sb, chunk, blk_in_sb); the edge stream is sorted by it.
    NSEG = NSB * N_CHUNK * SBB

    per_core_raw = []
    counts = np.zeros(NSEG, np.int64)
    for c in range(NC):
        m = (dst // NPC) == c
        s = src[m].astype(np.int64)
        d_l = (dst[m] - c * NPC).astype(np.int64)
        roll = (s - c * NPC) % N
        pseudo = (roll % 128) * NT_G + roll // 128
        chunk = (roll % 128) // CH_PARTS
        src_loc = pseudo - chunk * CHUNK_PSEUDO
        block = d_l // BLK
        slot = d_l % BLK
        sb = block // SBB
        blk_in = block % SBB
        seg = (sb * N_CHUNK + chunk) * SBB + blk_in
        cnt = np.bincount(seg, minlength=NSEG)
        counts = np.maximum(counts, cnt)
        per_core_raw.append((seg, src_loc, d_l, slot))

    P = ((counts + 127) // 128) * 128  # shared per-seg budgets
    # Guarantee every (sb, blk) has >= 1 tile so its PSUM region is written.
    for sb in range(NSB):
        for b in range(SBB):
            if sb * SBB + b >= NBLK:
                continue
            segs = [(sb * N_CHUNK + ch) * SBB + b for ch in range(N_CHUNK)]
            if P[segs].sum() == 0:
                P[segs[0]] = 128
    seg_off = np.concatenate([[0], np.cumsum(P)])
    PT = int(seg_off[-1])  # total padded edges
    TT = PT // 128  # total tiles

    # Per-tile metadata (shared geometry).
    tile_block = np.empty(TT, np.int64)  # global block id
    for g in range(NSEG):
        lo, hi = seg_off[g] // 128, seg_off[g + 1] // 128
        sb, rem = divmod(g, N_CHUNK * SBB)
        ch, b = divmod(rem, SBB)
        tile_block[lo:hi] = sb * SBB + b
    tile_sb = tile_block // SBB
    tile_reg = tile_block % SBB
    first = np.zeros(TT, bool)
    last = np.zeros(TT, bool)
    seen = {}
    for t in range(TT):
        if tile_block[t] not in seen:
            first[t] = True
        seen[tile_block[t]] = t
    for b, t in seen.items():
        last[t] = True
    # per-sb tile ranges
    sb_t0 = np.searchsorted(tile_sb, np.arange(NSB))
    sb_t1 = np.searchsorted(tile_sb, np.arange(NSB), side="right")
    T_SB_MAX = int((sb_t1 - sb_t0).max())

    geom = dict(P=P, seg_off=seg_off, PT=PT, TT=TT, tile_block=tile_block,
                tile_sb=tile_sb, tile_reg=tile_reg, first=first, last=last,
                sb_t0=sb_t0, sb_t1=sb_t1, T_SB_MAX=T_SB_MAX)

    per_core = []
    for c in range(NC):
        seg, src_loc, d_l, slot = per_core_raw[c]
        order = np.argsort(seg, kind="stable")
        # position within segment
        gs = np.full(PT, 0, np.int32)  # pad src idx: window row 0 (valid)
        gd = np.full(PT, ZROW, np.int32)  # pad dst idx: the zero row
        sl = np.full(PT, -1.0, np.float32)  # pad slot: one-hot row of zeros
        seg_sorted = seg[order]
        # rank within each seg
        boundaries = np.flatnonzero(np.r_[True, seg_sorted[1:] != seg_sorted[:-1]])
        seg_counts = np.diff(np.r_[boundaries, len(seg_sorted)])
        rank = np.arange(len(seg_sorted)) - np.repeat(boundaries, seg_counts)
        pos = seg_off[seg_sorted] + rank
        gs[pos] = src_loc[order]
        gd[pos] = d_l[order]
        sl[pos] = slot[order]

        # wrap per (sb, chunk) span for gsrc; per sb span for gdst
        gsrc_blocks, gdst_blocks = [], []
        for sb in range(NSB):
            base = sb * N_CHUNK * SBB
            sb_lo = seg_off[base]
            for ch in range(N_CHUNK):
                lo = seg_off[base + ch * SBB]
                hi = seg_off[base + (ch + 1) * SBB]
                n = int(hi - lo)
                if n:
                    gsrc_blocks.append(_wrap_idx(gs[lo:hi], n))
            sb_hi = seg_off[min(base + N_CHUNK * SBB, NSEG)]
            n = int(sb_hi - sb_lo)
            if n:
                gdst_blocks.append(_wrap_idx(gd[sb_lo:sb_hi], n))
        slots = sl.reshape(TT, 128).T.astype(ml_dtypes.bfloat16)
        per_core.append(dict(
            gsrc_idx=np.concatenate(gsrc_blocks, axis=1),
            gdst_idx=np.concatenate(gdst_blocks, axis=1),
            slots=np.ascontiguousarray(slots),
        ))
    return geom, per_core


def _build(h, W, src, dst):
    h = np.asarray(h, np.float32)
    W = np.asarray(W, np.float32)
    src = np.asarray(src).astype(np.int64)
    dst = np.asarray(dst).astype(np.int64)

    # Softmax shift: exp(e) can overflow fp32 for hot edges (e.g. self-loops
    # with |z|^2 > 88). exp(lrelu(e) - C) with a global C keeps every
    # exponent in range; alpha = ex/denom is exactly shift-invariant.
    z_host = h @ W.T
    e_max = 0.0
    for lo in range(0, len(src), 200000):
        sl = slice(lo, lo + 200000)
        e_max = max(e_max, float(
            np.einsum("ij,ij->i", z_host[src[sl]], z_host[dst[sl]]).max()))
    EXP_SHIFT = max(0.0, e_max - 40.0)

    geom, per_core = _plan(src, dst)
    P, seg_off, PT, TT = geom["P"], geom["seg_off"], geom["PT"], geom["TT"]
    sb_t0, sb_t1, T_SB_MAX = geom["sb_t0"], geom["sb_t1"], geom["T_SB_MAX"]
    tile_reg, first, last = geom["tile_reg"], geom["first"], geom["last"]

    # ---- host tensors ---------------------------------------------------
    hT = np.ascontiguousarray(h.T)  # [128, N]
    wT = np.ascontiguousarray(W.T).astype(np.float16)  # [128, 64]
    iota = np.tile(np.arange(128, dtype=np.float32), (128, 1)).astype(
        ml_dtypes.bfloat16)

    in_maps = []
    for c in range(NC):
        hp = np.zeros((IN_DIM, NROWS), np.float16)
        hp[:, :N] = np.roll(hT, -c * NPC, axis=1).astype(np.float16)
        im = dict(hT=hp, wT=wT, iota=iota, **per_core[c])
        in_maps.append(im)

    # ---- device program -------------------------------------------------
    # The tile framework round-robins Pool DMA insts over 8 DMASW sem lanes
    # in *scheduled* order, and each lane is locked to one SWDGE queue. To
    # spread gathers over NQ descriptor rings we build twice: pass 1 (all
    # queue 0) reveals each gather's lane; pass 2 sets queue = lane % NQ.
    NQ = 4
    DEBUG = bool(int(os.environ.get("GAT_DEBUG", "0")))
    NSB_RUN = int(os.environ.get("GAT_NSB", NSB))

    def _emit(queue_plan):
        gathers = []  # mybir instruction objects in emission order

        def _q():
            i = len(gathers)
            if queue_plan is not None and i < len(queue_plan):
                return int(queue_plan[i])
            return 0

        nc = bacc.Bacc(None, target_bir_lowering=False, debug=False,
                       num_swdge_queues=NQ)
        hT_d = nc.declare_dram_parameter("hT", [IN_DIM, NROWS], FP16, isOutput=False)
        wT_d = nc.declare_dram_parameter("wT", [IN_DIM, OUT_DIM], FP16, isOutput=False)
        iota_d = nc.declare_dram_parameter("iota", [128, 128], BF16, isOutput=False)
        gsrc_d = nc.declare_dram_parameter("gsrc_idx", list(in_maps[0]["gsrc_idx"].shape), I16, isOutput=False)
        gdst_d = nc.declare_dram_parameter("gdst_idx", list(in_maps[0]["gdst_idx"].shape), I16, isOutput=False)
        slots_d = nc.declare_dram_parameter("slots", [128, TT], BF16, isOutput=False)
        out_d = nc.declare_dram_parameter("out", [NBLK * BLK, OUT_DIM], F32, isOutput=True)
        if DEBUG:
            zchk_d = nc.declare_dram_parameter("z_chk", [256, OUT_DIM], F32, isOutput=True)
            aggdump_d = nc.declare_dram_parameter("agg_dump", [128, NBLK * VD], F32, isOutput=True)
            edump_d = nc.declare_dram_parameter("e_dump", [128, 48], F32, isOutput=True)
            exdump_d = nc.declare_dram_parameter("ex_dump", [128, 48], F32, isOutput=True)

        z_all = nc.dram_tensor("z_all", [128 * NT_G, OUT_DIM], F32)
        z_own = nc.dram_tensor("z_own", [ZOWN_ROWS, OUT_DIM], F32)

        QB = 8

        with tile.TileContext(nc) as tc:
            with tc.tile_pool(name="cst", bufs=1) as cpool:
                iota_t = cpool.tile([128, 128], BF16)
                nc.sync.dma_start(iota_t[:], iota_d[:])
                agg = cpool.tile([128, NBLK, VD], F32)
                shiftt = cpool.tile([128, 1], F32)
                nc.vector.memset(shiftt[:], -EXP_SHIFT)

                # ---------- phase A: z = h @ W^T -------------------------
                # 8 row-tiles accumulate into disjoint slices of one PSUM
                # bank (one start/stop group; start's lazy zero covers the
                # bank); one chunked ACT copy stages z for the DMA writes.
                with tc.tile_pool(name="w", bufs=1) as wpool, \
                     tc.tile_pool(name="hst", bufs=3) as hpool, \
                     tc.tile_pool(name="psA", bufs=4, space="PSUM") as pspool, \
                     tc.tile_pool(name="zst", bufs=1) as zpool:
                    wt = wpool.tile([IN_DIM, OUT_DIM], FP16)
                    nc.sync.dma_start(wt[:], wT_d[:])
                    z_all3 = z_all[:].rearrange("(p i) d -> p i d", p=128)
                    for i0 in range(0, NT_G, QB):
                        qb = min(QB, NT_G - i0)
                        hstage = hpool.tile([IN_DIM, QB * 128], FP16, tag="hstage")
                        nc.sync.dma_start(hstage[:, : qb * 128],
                                          hT_d[:, i0 * 128:(i0 + qb) * 128])
                        ps = pspool.tile([128, QB, OUT_DIM], F32)
                        for j in range(qb):
                            nc.tensor.matmul(ps[:, j, :],
                                             hstage[:, j * 128:(j + 1) * 128],
                                             wt[:], start=(j == 0),
                                             stop=(j == qb - 1))
                        zstage = zpool.tile([128, QB, OUT_DIM], F32,
                                            tag="zstage", bufs=3)
                        nc.scalar.activation(zstage[:, :qb, :], ps[:, :qb, :],
                                             AF.Copy)
                        nc.sync.dma_start(z_all3[:, i0:i0 + qb, :],
                                          zstage[:, :qb, :])
                        hi = min((i0 + qb) * 128, ZROW)
                        if i0 * 128 < ZROW:
                            qo = (hi - i0 * 128) // 128
                            zo_v = z_own[i0 * 128: hi, :].rearrange(
                                "(q p) d -> p q d", p=128)
                            nc.sync.dma_start(zo_v, zstage[:, :qo, :])
                    ztile0 = zpool.tile([128, OUT_DIM], F32, tag="zzero")
                    nc.vector.memset(ztile0[:], 0.0)
                    nc.sync.dma_start(z_own[ZROW:ZROW + 128, :], ztile0[:])

                # ---------- phase B: edge superbatches -------------------
                with tc.tile_pool(name="gat", bufs=2) as gpool, \
                     tc.tile_pool(name="sex", bufs=2) as spool, \
                     tc.tile_pool(name="ind", bufs=2) as ipool, \
                     tc.tile_pool(name="prd", bufs=2) as ppool, \
                     tc.tile_pool(name="val", bufs=2) as vpool, \
                     tc.tile_pool(name="sm", bufs=3) as smpool, \
                     tc.tile_pool(name="psB", bufs=2, space="PSUM") as psB, \
                     tc.tile_pool(name="ix", bufs=2) as xpool:

                    def issue_gathers(sb):
                        t0, t1 = int(sb_t0[sb]), int(sb_t1[sb])
                        Tsb = t1 - t0
                        Esb = Tsb * 128
                        base = sb * N_CHUNK * SBB
                        gs_off = int(seg_off[base]) // 16
                        gd_off = gs_off

                        zsrc = gpool.tile([128, T_SB_MAX, OUT_DIM], F32, tag="zsrc")
                        zdst = gpool.tile([128, T_SB_MAX, OUT_DIM], F32, tag="zdst")

                        igs = xpool.tile([128, T_SB_MAX * 8], I16, tag="igs")
                        nc.sync.dma_start(igs[:, : Esb // 16],
                                          gsrc_d[:, gs_off: gs_off + Esb // 16])
                        off = 0
                        for ch in range(N_CHUNK):
                            pcnt = int(P[base + ch * SBB: base + (ch + 1) * SBB].sum())
                            for o2 in range(0, pcnt, GMAX):
                                n2 = min(GMAX, pcnt - o2)
                                g = nc.gpsimd.dma_gather(
                                    zsrc[:, (off + o2) // 128:(off + o2 + n2) // 128, :],
                                    z_all[ch * CHUNK_PSEUDO:(ch + 1) * CHUNK_PSEUDO, :],
                                    igs[:, (off + o2) // 16:(off + o2 + n2) // 16],
                                    n2, n2, OUT_DIM, single_packet=False,
                                    queue_num=_q())
                                gathers.append(g.ins)
                            off += pcnt

                        igd = xpool.tile([128, T_SB_MAX * 8], I16, tag="igd")
                        nc.sync.dma_start(igd[:, : Esb // 16],
                                          gdst_d[:, gd_off: gd_off + Esb // 16])
                        for o2 in range(0, Esb, GMAX):
                            n2 = min(GMAX, Esb - o2)
                            g = nc.gpsimd.dma_gather(
                                zdst[:, o2 // 128:(o2 + n2) // 128, :], z_own[:],
                                igd[:, o2 // 16:(o2 + n2) // 16],
                                n2, n2, OUT_DIM, single_packet=False,
                                queue_num=_q())
                            gathers.append(g.ins)

                        slt = smpool.tile([128, T_SB_MAX], BF16, tag="slt")
                        nc.sync.dma_start(slt[:, :Tsb], slots_d[:, t0:t1])
                        return zsrc, zdst, slt

                    def compute(sb, tiles):
                        zsrc, zdst, slt = tiles
                        t0, t1 = int(sb_t0[sb]), int(sb_t1[sb])
                        Tsb = t1 - t0
                        sexp = spool.tile([128, T_SB_MAX, 128], BF16, tag="sexp")
                        nc.scalar.activation(
                            sexp[:, :Tsb, :],
                            slt[:, :Tsb, None].broadcast_to((128, Tsb, 128)),
                            AF.Copy)
                        ind = ipool.tile([128, T_SB_MAX, 128], BF16, tag="ind")
                        nc.vector.tensor_tensor(
                            ind[:, :Tsb, :],
                            iota_t[:, None, :].broadcast_to((128, Tsb, 128)),
                            sexp[:, :Tsb, :], op=ALU.is_equal)

                        prod = ppool.tile([128, T_SB_MAX, OUT_DIM], F32, tag="prod")
                        nc.vector.tensor_mul(prod[:, :Tsb, :], zsrc[:, :Tsb, :],
                                             zdst[:, :Tsb, :])
                        e = smpool.tile([128, T_SB_MAX], F32, tag="e")
                        nc.vector.tensor_reduce(e[:, :Tsb], prod[:, :Tsb, :],
                                                axis=mybir.AxisListType.X,
                                                op=ALU.add)
                        x1 = smpool.tile([128, T_SB_MAX], F32, tag="x1")
                        nc.scalar.activation(x1[:, :Tsb], e[:, :Tsb], AF.Exp,
                                             bias=shiftt[:])
                        x2 = smpool.tile([128, T_SB_MAX], F32, tag="x2")
                        nc.scalar.activation(x2[:, :Tsb], e[:, :Tsb], AF.Exp,
                                             scale=0.2, bias=shiftt[:])
                        ex = smpool.tile([128, T_SB_MAX], F32, tag="ex")
                        nc.vector.tensor_max(ex[:, :Tsb], x1[:, :Tsb], x2[:, :Tsb])

                        vals = vpool.tile([128, T_SB_MAX, VD], BF16, tag="vals")
                        nc.vector.tensor_mul(
                            vals[:, :Tsb, 0:OUT_DIM], zsrc[:, :Tsb, :],
                            ex[:, :Tsb, None].broadcast_to((128, Tsb, OUT_DIM)))
                        nc.vector.tensor_copy(vals[:, :Tsb, OUT_DIM], ex[:, :Tsb])

                        if DEBUG and sb == 0:
                            nc.sync.dma_start(edump_d[:], e[:, :48])
                            nc.sync.dma_start(exdump_d[:], ex[:, :48])

                        # one full PSUM bank (2KB zero region) per dst block:
                        # matmul start=True lazily zeroes the whole bank, so
                        # accumulation groups cannot share one.
                        aggps = psB.tile([128, SBB, 512], F32, tag="aggps")
                        for t in range(t0, t1):
                            r = int(tile_reg[t])
                            nc.tensor.matmul(aggps[:, r, 0:VD], ind[:, t - t0, :],
                                             vals[:, t - t0, :],
                                             start=bool(first[t]),
                                             stop=bool(last[t]))
                        nblk_sb = min(SBB, NBLK - sb * SBB)
                        nc.scalar.activation(
                            agg[:, sb * SBB: sb * SBB + nblk_sb, :],
                            aggps[:, :nblk_sb, 0:VD], AF.Copy)

                    pending = {}
                    for sb in range(NSB_RUN + 1):
                        if sb < NSB_RUN:
                            pending[sb] = issue_gathers(sb)
                        if sb >= 1:
                            compute(sb - 1, pending.pop(sb - 1))

                if DEBUG:
                    with tc.tile_pool(name="dbg", bufs=1) as dpool:
                        zt = dpool.tile([128, 2, OUT_DIM], F32)
                        nc.sync.dma_start(
                            zt[:], z_own[0:256, :].rearrange("(t p) c -> p t c", p=128))
                        nc.sync.dma_start(
                            zchk_d[:].rearrange("(t p) c -> p t c", p=128), zt[:])
                        nc.sync.dma_start(
                            aggdump_d[:].rearrange("p (b v) -> p b v", b=NBLK), agg[:])

                # ---------- phase D: normalize + elu ---------------------
                if NSB_RUN == NSB:
                    with tc.tile_pool(name="fin", bufs=1) as fpool:
                        d1 = fpool.tile([128, NBLK], F32)
                        nc.vector.tensor_scalar_add(d1[:], agg[:, :, OUT_DIM], 1e-30)
                        r = fpool.tile([128, NBLK], F32)
                        nc.vector.reciprocal(r[:], d1[:])
                        o64 = fpool.tile([128, NBLK, OUT_DIM], F32)
                        nc.vector.tensor_mul(
                            o64[:], agg[:, :, 0:OUT_DIM],
                            r[:, :, None].broadcast_to((128, NBLK, OUT_DIM)))
                        mn = fpool.tile([128, NBLK, OUT_DIM], F32)
                        nc.vector.tensor_scalar_min(mn[:], o64[:], 0.0)
                        emn = fpool.tile([128, NBLK, OUT_DIM], F32)
                        nc.scalar.activation(emn[:], mn[:], AF.Exp)
                        mx = fpool.tile([128, NBLK, OUT_DIM], F32)
                        nc.vector.tensor_scalar_max(mx[:], o64[:], 0.0)
                        res = fpool.tile([128, NBLK, OUT_DIM], F32)
                        nc.vector.scalar_tensor_tensor(res[:], in0=emn[:],
                                                       scalar=-1.0, in1=mx[:],
                                                       op0=ALU.add, op1=ALU.add)
                        out_v = out_d[:].rearrange("(b p) c -> p b c", p=128)
                        nc.sync.dma_start(out_v, res[:])

        nc.finalize()
        return nc, gathers

    from concourse.tile_sem_assignment import PROC_NAME_TO_IDX
    idx_to_lane = {PROC_NAME_TO_IDX[f"DMASW{i}"]: i for i in range(8)}

    def _lanes(gathers):
        out = []
        for g in gathers:
            proc = getattr(g, "bass_scheduled_proc", None)
            out.append(idx_to_lane.get(proc, -1))
        return out

    nc1, g1 = _emit(None)
    lanes = _lanes(g1)
    if all(l >= 0 for l in lanes):
        plan = [l % NQ for l in lanes]
        nc2, g2 = _emit(plan)
        lanes2 = _lanes(g2)
        if all(l >= 0 and l % NQ == q for l, q in zip(lanes2, plan)):
            return nc2, in_maps, dict(NC=NC, NPC=NPC)
    return nc1, in_maps, dict(NC=NC, NPC=NPC)


def kernel(h, W, src, dst):
    global LAST_RESULTS, LAST_BUILD
    nc, in_maps, meta = _build(h, W, src, dst)
    LAST_BUILD = (nc, in_maps, meta)
    results = run_bass_kernel_spmd(
        nc, in_maps, core_ids=list(range(meta["NC"])),
        trace=bool(int(os.environ.get("GAT_TRACE", "0"))),
    )
    LAST_RESULTS = results
    out = np.concatenate(
        [results.results[c]["out"][:meta["NPC"]] for c in range(meta["NC"])], axis=0)
    return out.astype(np.float32)



# revision 2
# speedup vs baseline: 1.5396x; 1.5396x over previous
"""GAT layer on 8 Trainium2 NeuronCores — identity-slot scheme.

Per core c (SPMD, per-core input maps, shared geometry):
  - Core c owns dst nodes [c*NPC, (c+1)*NPC). Own nodes are assigned to
    (block, slot) positions by a host-side packing that groups nodes with
    similar per-chunk in-degree vectors; the assignment is folded into the
    per-core hT column permutation so phase A produces z_own (SBUF,
    [slot, block, 64] fp16) directly — no on-chip permutation.
  - Sources are assigned to one of 4 gather-window chunks by a greedy
    per-dst balance (keeps per-(block,chunk) max slot-multiplicity ~deg/4),
    folded into the same hT column permutation.
  - Edge stream: per superbatch of blocks, tiles ordered (chunk, block,
    round). A tile holds at most one edge per slot, AT its slot position
    (identity layout) — so no one-hot build, no dst gather, no slot upload:
      prod = zsrc * z_own[block]  (broadcast), e = rowsum, ex = leaky-exp,
      vals = zsrc * ex, psum[block] += vals via PE matmul with identity A.
  - Pads gather a zero z row => vals 0; their exp(-C) denominator
    contribution is subtracted at the end using the device's own exp(-C)
    and a host pad-count tensor.
  - Softmax shift C = e_max - 10 keeps ex <= e^10 in fp16 range; terms
    with alpha < ~e-12 underflow fp16 to 0 (negligible).
"""

import os
import sys

sys.path.insert(0, "/opt/trn_rl_repo")

import numpy as np

import concourse.bacc as bacc
import concourse.mybir as mybir
import concourse.tile as tile
from concourse.bass_utils import run_bass_kernel_spmd

F32 = mybir.dt.float32
BF16 = mybir.dt.bfloat16
FP16 = mybir.dt.float16
I16 = mybir.dt.int16
AF = mybir.ActivationFunctionType
ALU = mybir.AluOpType

LAST_RESULTS = None
LAST_BUILD = None

N = 100000
E_TOT = 1600000
IN_DIM = 128
OUT_DIM = 64
NC = 8
NPC = N // NC  # 12500
NT_G = (N + 127) // 128  # 782 GEMM row tiles
NROWS = NT_G * 128  # 100096
N_CHUNK = 4
CH_PARTS = 32
CHUNK_PSEUDO = CH_PARTS * NT_G  # 25024
BLK = 128
NBLK = (NPC + BLK - 1) // BLK  # 98
VD = OUT_DIM + 1  # 65
T_SB = 128  # tiles per superbatch (target)
GMAX = 8192
QB = 8  # GEMM row tiles per stage


def _wrap_idx(idx, budget):
    """[n] int -> [128, budget//16] int16 wrapped + replicated (q7 layout)."""
    a = np.zeros(budget, np.int16)
    a[: len(idx)] = idx.astype(np.int16)
    w = a.reshape(budget // 16, 16).T.copy()
    return np.tile(w, (8, 1))


def _plan(src, dst):
    """Host planning. Returns (geom, per_core)."""
    # ---------------- per-core: chunk assign + packing ----------------
    per_core_raw = []
    dq_all = []
    for c in range(NC):
        m = (dst // NPC) == c
        u = src[m].astype(np.int64)
        v = (dst[m] - c * NPC).astype(np.int64)
        deg = np.bincount(v, minlength=NPC)

        # greedy chunk assignment of src nodes (per-dst balance)
        o = np.argsort(u, kind="stable")
        us, vs = u[o], v[o]
        uniq, starts = np.unique(us, return_index=True)
        ends = np.r_[starts[1:], len(us)]
        cnt = ends - starts
        proc = np.argsort(-cnt, kind="stable")
        node_outcnt = np.zeros(N, np.int64)
        node_outcnt[uniq] = cnt
        dq = np.zeros((NPC, 4), np.int32)
        node_chunk = np.full(N, -1, np.int32)
        # chunk capacities for non-own src nodes (columns >= 12544 with
        # matching partition group), minus 1 reserved zero column each
        cols = np.arange(NBLK * BLK, NROWS)
        colchunk = (cols % 128) // CH_PARTS
        cap = np.bincount(colchunk, minlength=4) - 1
        own_lo, own_hi = c * NPC, (c + 1) * NPC
        used = np.zeros(4, np.int64)
        for i in proc:
            nd = int(uniq[i])
            vv = vs[starts[i]:ends[i]]
            score = dq[vv].sum(axis=0).astype(np.float64)
            if not (own_lo <= nd < own_hi):
                score[used >= cap] = np.inf
            q = int(np.argmin(score))
            node_chunk[nd] = q
            np.add.at(dq, (vv, q), 1)
            if not (own_lo <= nd < own_hi):
                used[q] += 1

        # pack own dst nodes into blocks by chunk-degree vector
        order_v = np.lexsort((-dq[:, 3], -dq[:, 2], -dq[:, 1], -dq[:, 0],
                              -dq.max(1)))
        # within each block, hand out slots so a node's partition group
        # (slot//32) matches its own src-chunk preference where possible;
        # spilled nodes get their chunk forced to the slot's group.
        slot_of = np.empty(NPC, np.int64)
        uniq_set = set(uniq.tolist())
        own_lo2 = own_lo
        for b in range((NPC + BLK - 1) // BLK):
            grp = order_v[b * BLK:(b + 1) * BLK]
            prefs = node_chunk[grp + own_lo2]
            taken = np.zeros(4, np.int64)
            limit = np.bincount(
                np.arange(len(grp)) * 0, minlength=1)  # placeholder
            # last block may be short; group capacity is slots present
            nslots = len(grp) if False else BLK
            slot_used = np.zeros(BLK, bool)
            order_in = np.argsort(-cnt_of_node[grp]) if False else \
                np.argsort(-node_outcnt[grp + own_lo2], kind="stable")
            assign = np.full(len(grp), -1, np.int64)
            cap4 = np.array([32, 32, 32, 32])
            # cap by actual slots available in this block (always 128 slots,
            # last block just has fewer nodes)
            for j in order_in:
                q = prefs[j]
                if q >= 0 and taken[q] < cap4[q]:
                    base = q * 32 + taken[q]
                    assign[j] = base
                    taken[q] += 1
            for j in order_in:
                if assign[j] >= 0:
                    continue
                q = int(np.argmin(taken - cap4))
                # first group with room
                for q2 in np.argsort(taken - cap4):
                    if taken[q2] < cap4[q2]:
                        q = int(q2)
                        break
                assign[j] = q * 32 + taken[q]
                taken[q] += 1
                nd = int(grp[j]) + own_lo2
                if node_chunk[nd] != q and nd in uniq_set:
                    i = np.searchsorted(uniq, nd)
                    vv = vs[starts[i]:ends[i]]
                    if node_chunk[nd] >= 0:
                        np.add.at(dq, (vv, node_chunk[nd]), -1)
                    np.add.at(dq, (vv, q), 1)
                    node_chunk[nd] = q
                elif node_chunk[nd] != q:
                    node_chunk[nd] = q
            slot_of[grp] = b * BLK + assign

        # rebalance non-own src nodes around the (now fixed) own chunks
        dq2 = np.zeros((NPC, 4), np.int32)
        own_mask_u = (uniq >= own_lo) & (uniq < own_hi)
        for i in np.flatnonzero(own_mask_u):
            np.add.at(dq2, (vs[starts[i]:ends[i]], node_chunk[uniq[i]]), 1)
        used = np.zeros(4, np.int64)
        for i in proc:
            nd = int(uniq[i])
            if own_lo <= nd < own_hi:
                continue
            vv = vs[starts[i]:ends[i]]
            score = dq2[vv].sum(axis=0).astype(np.float64)
            score[used >= cap] = np.inf
            q = int(np.argmin(score))
            node_chunk[nd] = q
            np.add.at(dq2, (vv, q), 1)
            used[q] += 1
        dq = dq2

        per_core_raw.append(dict(u=u, v=v, deg=deg, dq=dq, slot_of=slot_of,
                                 node_chunk=node_chunk, cap=cap))
        dq_all.append(dq)

    # ---------------- shared budgets R[b, q] = max over cores ----------
    R = np.zeros((NBLK, 4), np.int64)
    for c in range(NC):
        dq = dq_all[c]
        slot_of = per_core_raw[c]["slot_of"]
        dqb = np.zeros((NBLK * BLK, 4), np.int64)
        dqb[slot_of] = dq
        R = np.maximum(R, dqb.reshape(NBLK, BLK, 4).max(axis=1))
    # every block needs >= 1 tile so its PSUM region gets written
    empty = R.sum(axis=1) == 0
    R[empty, 0] = 1

    # superbatches: consecutive blocks while tile budget fits T_SB
    blk_tiles = R.sum(axis=1)
    sbs = []
    cur = []
    cur_t = 0
    for b in range(NBLK):
        t = int(blk_tiles[b])
        if cur and cur_t + t > T_SB:
            sbs.append(cur)
            cur, cur_t = [], 0
        cur.append(b)
        cur_t += t
    if cur:
        sbs.append(cur)

    # tile layout: for sb: for q: for b in sb: R[b, q] tiles
    tile_of = {}  # (b, q) -> (tile_start)
    sb_meta = []  # per sb: (t0, tiles, [(q, b, t0_rel, R_bq)...], [(q, t0_rel, ntiles)])
    t_acc = 0
    for blist in sbs:
        runs = []
        qspans = []
        t_rel = 0
        for q in range(4):
            q0 = t_rel
            for b in blist:
                r = int(R[b, q])
                if r == 0:
                    continue
                tile_of[(b, q)] = t_acc + t_rel
                runs.append((q, b, t_rel, r))
                t_rel += r
            if t_rel > q0:
                qspans.append((q, q0, t_rel - q0))
        sb_meta.append(dict(t0=t_acc, tiles=t_rel, blocks=list(blist),
                            runs=runs, qspans=qspans))
        t_acc += t_rel
    TT = t_acc
    POS = TT * 128

    geom = dict(R=R, sbs=sbs, sb_meta=sb_meta, TT=TT, POS=POS)

    # ---------------- per-core edge placement ----------------
    per_core = []
    for c in range(NC):
        pc = per_core_raw[c]
        u, v, deg = pc["u"], pc["v"], pc["deg"]
        slot_of, node_chunk = pc["slot_of"], pc["node_chunk"]

        # column assignment: own at slot, non-own greedy by chunk
        col_of = np.full(N, -1, np.int64)
        own_ids = np.arange(c * NPC, (c + 1) * NPC)
        col_of[own_ids] = slot_of  # block*128+slot == slot_of index layout
        cols = np.arange(NBLK * BLK, NROWS)
        colchunk = (cols % 128) // CH_PARTS
        zero_col = np.empty(4, np.int64)
        free_cols = []
        for q in range(4):
            qc = cols[colchunk == q]
            zero_col[q] = qc[-1]  # reserved zero column (no node)
            free_cols.append(qc[:-1])
        non_own = np.setdiff1d(np.arange(N), own_ids, assume_unique=True)
        nq = node_chunk[non_own]
        # nodes with chunk -1 (no out-edges here): fill leftover capacity
        ptr = [0, 0, 0, 0]
        for q in range(4):
            sel = non_own[nq == q]
            fc = free_cols[q]
            col_of[sel] = fc[: len(sel)]
            ptr[q] = len(sel)
        rest = non_own[nq == -1]
        ri = 0
        for q in range(4):
            fc = free_cols[q]
            room = len(fc) - ptr[q]
            take = min(room, len(rest) - ri)
            if take > 0:
                col_of[rest[ri:ri + take]] = fc[ptr[q]:ptr[q] + take]
                ri += take
        assert ri == len(rest), "column capacity exhausted"

        # window-local gather index (maps a column id to its in-window row)
        def col_local(g):
            pseudo = (g % 128) * NT_G + g // 128
            return pseudo - ((g % 128) // CH_PARTS) * CHUNK_PSEUDO

        local = col_local(col_of)

        # edge ranks within (v, q)
        qe = node_chunk[u]
        key = v * 4 + qe
        o = np.argsort(key, kind="stable")
        ks = key[o]
        b0 = np.flatnonzero(np.r_[True, ks[1:] != ks[:-1]])
        cnt2 = np.diff(np.r_[b0, len(ks)])
        rank = np.arange(len(ks)) - np.repeat(b0, cnt2)
        # position per edge
        bfull = slot_of[v[o]]
        blkid = bfull // BLK
        slot = bfull % BLK
        tbase = np.array([tile_of[(int(bb), int(qq))]
                          for bb, qq in zip(blkid, ks % 4)], np.int64)
        posn = (tbase + rank) * 128 + slot

        gs = np.empty(POS, np.int32)
        # pads: per tile the chunk is known; fill with zero col of that chunk
        padfill = np.empty(TT, np.int32)
        for sbm in sb_meta:
            for (q, b, t_rel, r) in sbm["runs"]:
                padfill[sbm["t0"] + t_rel: sbm["t0"] + t_rel + r] = \
                    col_local(zero_col[q])
        gs[:] = np.repeat(padfill, 128)
        gs[posn] = local[u[o]].astype(np.int32)

        # wrap per (sb, q) span
        blocks_w = []
        for sbm in sb_meta:
            t0 = sbm["t0"]
            for (q, q0, ntiles) in sbm["qspans"]:
                lo = (t0 + q0) * 128
                hi = lo + ntiles * 128
                blocks_w.append(_wrap_idx(gs[lo:hi], hi - lo))
        gsrc_idx = np.concatenate(blocks_w, axis=1)

        # pad counts per (slot s, block b): sum_q R[b, q] - deg(node(b, s))
        degfull = np.zeros(NBLK * BLK, np.int64)
        degfull[slot_of] = deg
        padcnt = (R.sum(axis=1)[None, :] -
                  degfull.reshape(NBLK, BLK).T).astype(np.float32)

        per_core.append(dict(gsrc_idx=gsrc_idx, padcnt=padcnt,
                             col_of=col_of, slot_of=slot_of))
    return geom, per_core


def _build(h, W, src, dst):
    h = np.asarray(h, np.float32)
    W = np.asarray(W, np.float32)
    src = np.asarray(src).astype(np.int64)
    dst = np.asarray(dst).astype(np.int64)

    # softmax shift: C = max(0, e_max - 40); bf16 vals hold ex <= e^40.
    z_host = h @ W.T
    e_max = 0.0
    for lo in range(0, len(src), 200000):
        sl = slice(lo, lo + 200000)
        e_max = max(e_max, float(
            np.einsum("ij,ij->i", z_host[src[sl]], z_host[dst[sl]]).max()))
    EXP_SHIFT = max(0.0, e_max - 40.0)

    geom, per_core = _plan(src, dst)
    sb_meta, TT, POS = geom["sb_meta"], geom["TT"], geom["POS"]
    NSB = len(sb_meta)
    T_MAX = max(s["tiles"] for s in sb_meta)

    # ---- host tensors ---------------------------------------------------
    hT = h.T  # [128, N]
    wT = np.ascontiguousarray(W.T).astype(np.float16)
    import ml_dtypes
    ident = np.eye(128, dtype=ml_dtypes.bfloat16)

    in_maps = []
    for c in range(NC):
        hp = np.zeros((IN_DIM, NROWS), np.float16)
        col = per_core[c]["col_of"]
        hp[:, col] = hT.astype(np.float16)
        im = dict(hT=hp, wT=wT, ident=ident,
                  gsrc_idx=per_core[c]["gsrc_idx"],
                  padcnt=per_core[c]["padcnt"])
        in_maps.append(im)

    NQ = 4
    NSB_RUN = int(os.environ.get("GAT_NSB", NSB))

    def _emit(queue_plan):
        gathers = []

        def _q():
            i = len(gathers)
            if queue_plan is not None and i < len(queue_plan):
                return int(queue_plan[i])
            return 0

        nc = bacc.Bacc(None, target_bir_lowering=False, debug=False,
                       num_swdge_queues=NQ)
        hT_d = nc.declare_dram_parameter("hT", [IN_DIM, NROWS], FP16, isOutput=False)
        wT_d = nc.declare_dram_parameter("wT", [IN_DIM, OUT_DIM], FP16, isOutput=False)
        ident_d = nc.declare_dram_parameter("ident", [128, 128], BF16, isOutput=False)
        gsrc_d = nc.declare_dram_parameter("gsrc_idx", list(in_maps[0]["gsrc_idx"].shape), I16, isOutput=False)
        padcnt_d = nc.declare_dram_parameter("padcnt", [128, NBLK], F32, isOutput=False)
        out_d = nc.declare_dram_parameter("out", [128 * NBLK, OUT_DIM], F32, isOutput=True)

        z_all = nc.dram_tensor("z_all", [128 * NT_G, 128], FP16)

        with tile.TileContext(nc) as tc:
            with tc.tile_pool(name="cst", bufs=1) as cpool:
                ident_t = cpool.tile([128, 128], BF16)
                nc.sync.dma_start(ident_t[:], ident_d[:])
                padcnt_t = cpool.tile([128, NBLK], F32)
                nc.sync.dma_start(padcnt_t[:], padcnt_d[:])
                z_own = cpool.tile([128, NBLK, OUT_DIM], FP16)
                agg = cpool.tile([128, NBLK, VD], F32)
                shiftt = cpool.tile([128, 1], F32)
                nc.vector.memset(shiftt[:], -EXP_SHIFT)

                # ---------- phase A: z = h @ W^T -------------------------
                with tc.tile_pool(name="w", bufs=1) as wpool, \
                     tc.tile_pool(name="hst", bufs=3) as hpool, \
                     tc.tile_pool(name="psA", bufs=4, space="PSUM") as pspool, \
                     tc.tile_pool(name="zst", bufs=3) as zpool:
                    wt = wpool.tile([IN_DIM, OUT_DIM], FP16)
                    nc.sync.dma_start(wt[:], wT_d[:])
                    z_all3 = z_all[:].rearrange("(p i) d -> p i d", p=128)
                    for i0 in range(0, NT_G, QB):
                        qb = min(QB, NT_G - i0)
                        hstage = hpool.tile([IN_DIM, QB * 128], FP16, tag="hstage")
                        nc.sync.dma_start(hstage[:, : qb * 128],
                                          hT_d[:, i0 * 128:(i0 + qb) * 128])
                        ps = pspool.tile([128, QB, OUT_DIM], F32)
                        for j in range(qb):
                            nc.tensor.matmul(ps[:, j, :],
                                             hstage[:, j * 128:(j + 1) * 128],
                                             wt[:], start=(j == 0),
                                             stop=(j == qb - 1))
                        zstage = zpool.tile([128, QB, 128], FP16,
                                            tag="zstage")
                        nc.vector.memset(zstage[:, :, OUT_DIM:128], 0.0)
                        nc.scalar.activation(zstage[:, :qb, 0:OUT_DIM],
                                             ps[:, :qb, :], AF.Copy)
                        nc.sync.dma_start(z_all3[:, i0:i0 + qb, :],
                                          zstage[:, :qb, :])
                        if i0 < NBLK:
                            qo = min(qb, NBLK - i0)
                            nc.vector.tensor_copy(z_own[:, i0:i0 + qo, :],
                                                  zstage[:, :qo, 0:OUT_DIM])

                # ---------- phase B: edge superbatches -------------------
                with tc.tile_pool(name="gat", bufs=2) as gpool, \
                     tc.tile_pool(name="pv", bufs=2) as pvpool, \
                     tc.tile_pool(name="sm", bufs=2) as smpool, \
                     tc.tile_pool(name="psB", bufs=8, space="PSUM") as psB, \
                     tc.tile_pool(name="ix", bufs=2) as xpool:

                    def issue(si):
                        sbm = sb_meta[si]
                        t0, Ts = sbm["t0"], sbm["tiles"]
                        zsrc = gpool.tile([128, T_MAX, 128], FP16, tag="zsrc")
                        igs = xpool.tile([128, T_MAX * 8], I16, tag="igs")
                        nc.sync.dma_start(igs[:, : Ts * 8],
                                          gsrc_d[:, t0 * 8: (t0 + Ts) * 8])
                        for (q, q0, ntiles) in sbm["qspans"]:
                            n = ntiles * 128
                            for o2 in range(0, n, GMAX):
                                n2 = min(GMAX, n - o2)
                                g = nc.gpsimd.dma_gather(
                                    zsrc[:, q0 + o2 // 128: q0 + (o2 + n2) // 128, :],
                                    z_all[q * CHUNK_PSEUDO:(q + 1) * CHUNK_PSEUDO, :],
                                    igs[:, q0 * 8 + o2 // 16: q0 * 8 + (o2 + n2) // 16],
                                    n2, n2, 128, single_packet=False,
                                    queue_num=_q())
                                gathers.append(g.ins)
                        return zsrc

                    def compute(si, zsrc):
                        sbm = sb_meta[si]
                        Ts = sbm["tiles"]
                        prod = pvpool.tile([128, T_MAX, OUT_DIM], FP16,
                                           tag="prod")
                        vals = pvpool.tile([128, T_MAX, VD], BF16, tag="vals")
                        for (q, b, t_rel, r) in sbm["runs"]:
                            nc.vector.tensor_mul(
                                prod[:, t_rel:t_rel + r, :],
                                zsrc[:, t_rel:t_rel + r, 0:OUT_DIM],
                                z_own[:, b, None, :].broadcast_to(
                                    (128, r, OUT_DIM)))
                        # halving adds then reduce (fp16 partials of <=4
                        # products stay accurate)
                        nc.vector.tensor_add(prod[:, :Ts, 0:32],
                                             prod[:, :Ts, 0:32],
                                             prod[:, :Ts, 32:64])
                        nc.vector.tensor_add(prod[:, :Ts, 0:16],
                                             prod[:, :Ts, 0:16],
                                             prod[:, :Ts, 16:32])
                        e = smpool.tile([128, T_MAX], F32, tag="e")
                        nc.vector.tensor_reduce(e[:, :Ts], prod[:, :Ts, 0:16],
                                                axis=mybir.AxisListType.X,
                                                op=ALU.add)
                        x1 = smpool.tile([128, T_MAX], F32, tag="x1")
                        nc.scalar.activation(x1[:, :Ts], e[:, :Ts], AF.Exp,
                                             bias=shiftt[:])
                        x2 = smpool.tile([128, T_MAX], F32, tag="x2")
                        nc.scalar.activation(x2[:, :Ts], e[:, :Ts], AF.Exp,
                                             scale=0.2, bias=shiftt[:])
                        ex = smpool.tile([128, T_MAX], F32, tag="ex")
                        nc.vector.tensor_max(ex[:, :Ts], x1[:, :Ts], x2[:, :Ts])
                        nc.vector.tensor_copy(vals[:, :Ts, OUT_DIM],
                                              ex[:, :Ts])
                        # exB broadcast into vals[..0:64], then in-place
                        # vals = zsrc * exB
                        nc.scalar.activation(
                            vals[:, :Ts, 0:OUT_DIM],
                            ex[:, :Ts, None].broadcast_to((128, Ts, OUT_DIM)),
                            AF.Copy)
                        nc.vector.tensor_mul(vals[:, :Ts, 0:OUT_DIM],
                                             zsrc[:, :Ts, 0:OUT_DIM],
                                             vals[:, :Ts, 0:OUT_DIM])
                        # per-block PSUM accumulate + flush
                        tiles_b = {}
                        for (q, b, t_rel, r) in sbm["runs"]:
                            tiles_b.setdefault(b, []).extend(
                                range(t_rel, t_rel + r))
                        for b, tl in tiles_b.items():
                            psb = psB.tile([128, 512], F32, tag="psb")
                            for i, t in enumerate(tl):
                                nc.tensor.matmul(
                                    psb[:, 0:VD], ident_t[:],
                                    vals[:, t, 0:VD],
                                    start=(i == 0), stop=(i == len(tl) - 1))
                            nc.scalar.activation(agg[:, b, :], psb[:, 0:VD],
                                                 AF.Copy)

                    pending = {}
                    for si in range(NSB_RUN + 1):
                        if si < NSB_RUN:
                            pending[si] = issue(si)
                        if si >= 1:
                            compute(si - 1, pending.pop(si - 1))

                # ---------- phase C: pad fix + normalize + elu -----------
                if NSB_RUN == NSB:
                    with tc.tile_pool(name="fin", bufs=1) as fpool:
                        pexn0 = fpool.tile([128, 1], F32)
                        nc.scalar.activation(pexn0[:], shiftt[:], AF.Exp)
                        pexn16 = fpool.tile([128, 1], BF16)
                        nc.vector.tensor_copy(pexn16[:], pexn0[:])
                        pexn = fpool.tile([128, 1], F32)
                        nc.vector.tensor_copy(pexn[:], pexn16[:])
                        padsub = fpool.tile([128, NBLK], F32)
                        nc.vector.tensor_scalar(padsub[:], padcnt_t[:],
                                                pexn[:], None, op0=ALU.mult)
                        d1 = fpool.tile([128, NBLK], F32)
                        nc.vector.tensor_sub(d1[:], agg[:, :, OUT_DIM],
                                             padsub[:])
                        d2 = fpool.tile([128, NBLK], F32)
                        nc.vector.tensor_scalar_add(d2[:], d1[:], 1e-30)
                        rcp = fpool.tile([128, NBLK], F32)
                        nc.vector.reciprocal(rcp[:], d2[:])
                        HB = NBLK // 2  # 49
                        for h0 in range(0, NBLK, HB):
                            o64 = fpool.tile([128, HB, OUT_DIM], F32, tag="o64")
                            nc.vector.tensor_mul(
                                o64[:], agg[:, h0:h0 + HB, 0:OUT_DIM],
                                rcp[:, h0:h0 + HB, None].broadcast_to(
                                    (128, HB, OUT_DIM)))
                            mn = fpool.tile([128, HB, OUT_DIM], F32, tag="mn")
                            nc.vector.tensor_scalar_min(mn[:], o64[:], 0.0)
                            emn = fpool.tile([128, HB, OUT_DIM], F32, tag="emn")
                            nc.scalar.activation(emn[:], mn[:], AF.Exp)
                            mx = fpool.tile([128, HB, OUT_DIM], F32, tag="mx")
                            nc.vector.tensor_scalar_max(mx[:], o64[:], 0.0)
                            res = fpool.tile([128, HB, OUT_DIM], F32, tag="res")
                            nc.vector.scalar_tensor_tensor(
                                res[:], in0=emn[:], scalar=-1.0, in1=mx[:],
                                op0=ALU.add, op1=ALU.add)
                            out_v = out_d[:].rearrange(
                                "(p b) c -> p b c", p=128)[:, h0:h0 + HB, :]
                            nc.sync.dma_start(out_v, res[:])

        nc.finalize()
        return nc, gathers

    from concourse.tile_sem_assignment import PROC_NAME_TO_IDX
    idx_to_lane = {PROC_NAME_TO_IDX[f"DMASW{i}"]: i for i in range(8)}

    def _lanes(gathers):
        out = []
        for g in gathers:
            proc = getattr(g, "bass_scheduled_proc", None)
            out.append(idx_to_lane.get(proc, -1))
        return out

    nc1, g1 = _emit(None)
    lanes = _lanes(g1)
    meta = dict(NC=NC, NPC=NPC, per_core=per_core)
    if all(l >= 0 for l in lanes):
        plan = [l % NQ for l in lanes]
        nc2, g2 = _emit(plan)
        lanes2 = _lanes(g2)
        if all(l >= 0 and l % NQ == q for l, q in zip(lanes2, plan)):
            return nc2, in_maps, meta
    return nc1, in_maps, meta


def kernel(h, W, src, dst):
    global LAST_RESULTS, LAST_BUILD
    nc, in_maps, meta = _build(h, W, src, dst)
    LAST_BUILD = (nc, in_maps, meta)
    results = run_bass_kernel_spmd(
        nc, in_maps, core_ids=list(range(meta["NC"])),
        trace=bool(int(os.environ.get("GAT_TRACE", "0"))),
    )
    LAST_RESULTS = results
    out = np.empty((N, OUT_DIM), np.float32)
    for c in range(meta["NC"]):
        arr = results.results[c]["out"].reshape(128, NBLK, OUT_DIM)
        slot_of = meta["per_core"][c]["slot_of"]
        b = slot_of // BLK
        s = slot_of % BLK
        out[c * NPC:(c + 1) * NPC] = arr[s, b, :]
    return out.astype(np.float32)


# revision 3
# speedup vs baseline: 1.6380x; 1.0639x over previous
"""GAT layer on 8 Trainium2 NeuronCores — identity-slot scheme.

Per core c (SPMD, per-core input maps, shared geometry):
  - Core c owns dst nodes [c*NPC, (c+1)*NPC). Own nodes are assigned to
    (block, slot) positions by a host-side packing that groups nodes with
    similar per-chunk in-degree vectors; the assignment is folded into the
    per-core hT column permutation so phase A produces z_own (SBUF,
    [slot, block, 64] fp16) directly — no on-chip permutation.
  - Sources are assigned to one of 4 gather-window chunks by a greedy
    per-dst balance (keeps per-(block,chunk) max slot-multiplicity ~deg/4),
    folded into the same hT column permutation.
  - Edge stream: per superbatch of blocks, tiles ordered (chunk, block,
    round). A tile holds at most one edge per slot, AT its slot position
    (identity layout) — so no one-hot build, no dst gather, no slot upload:
      prod = zsrc * z_own[block]  (broadcast), e = rowsum, ex = leaky-exp,
      vals = zsrc * ex, psum[block] += vals via PE matmul with identity A.
  - Pads gather a zero z row => vals 0; their exp(-C) denominator
    contribution is subtracted at the end using the device's own exp(-C)
    and a host pad-count tensor.
  - Softmax shift C = e_max - 10 keeps ex <= e^10 in fp16 range; terms
    with alpha < ~e-12 underflow fp16 to 0 (negligible).
"""

import os
import sys

sys.path.insert(0, "/opt/trn_rl_repo")

import numpy as np

import concourse.bacc as bacc
import concourse.mybir as mybir
import concourse.tile as tile
from concourse.bass_utils import run_bass_kernel_spmd

F32 = mybir.dt.float32
BF16 = mybir.dt.bfloat16
FP16 = mybir.dt.float16
I16 = mybir.dt.int16
AF = mybir.ActivationFunctionType
ALU = mybir.AluOpType

LAST_RESULTS = None
LAST_BUILD = None

N = 100000
E_TOT = 1600000
IN_DIM = 128
OUT_DIM = 64
NC = 8
NPC = N // NC  # 12500
NT_G = (N + 127) // 128  # 782 GEMM row tiles
NROWS = NT_G * 128  # 100096
N_CHUNK = 4
CH_PARTS = 32
CHUNK_PSEUDO = CH_PARTS * NT_G  # 25024
BLK = 128
NBLK = (NPC + BLK - 1) // BLK  # 98
VD = OUT_DIM + 1  # 65
T_SB = 128  # tiles per superbatch (target)
GMAX = 8192
QB = 8  # GEMM row tiles per stage


def _wrap_idx(idx, budget):
    """[n] int -> [128, budget//16] int16 wrapped + replicated (q7 layout)."""
    a = np.zeros(budget, np.int16)
    a[: len(idx)] = idx.astype(np.int16)
    w = a.reshape(budget // 16, 16).T.copy()
    return np.tile(w, (8, 1))


def _plan(src, dst):
    """Host planning. Returns (geom, per_core)."""
    # ---------------- per-core: chunk assign + packing ----------------
    per_core_raw = []
    dq_all = []
    for c in range(NC):
        m = (dst // NPC) == c
        u = src[m].astype(np.int64)
        v = (dst[m] - c * NPC).astype(np.int64)
        deg = np.bincount(v, minlength=NPC)

        # greedy chunk assignment of src nodes (per-dst balance)
        o = np.argsort(u, kind="stable")
        us, vs = u[o], v[o]
        uniq, starts = np.unique(us, return_index=True)
        ends = np.r_[starts[1:], len(us)]
        cnt = ends - starts
        proc = np.argsort(-cnt, kind="stable")
        node_outcnt = np.zeros(N, np.int64)
        node_outcnt[uniq] = cnt
        dq = np.zeros((NPC, 4), np.int32)
        node_chunk = np.full(N, -1, np.int32)
        # chunk capacities: all columns, minus 1 reserved zero column each
        cols = np.arange(NROWS)
        colchunk = (cols % 128) // CH_PARTS
        cap = np.bincount(colchunk, minlength=4) - 1
        own_lo, own_hi = c * NPC, (c + 1) * NPC
        used = np.zeros(4, np.int64)
        for i in proc:
            nd = int(uniq[i])
            vv = vs[starts[i]:ends[i]]
            score = dq[vv].sum(axis=0).astype(np.float64)
            score[used >= cap] = np.inf
            q = int(np.argmin(score))
            node_chunk[nd] = q
            np.add.at(dq, (vv, q), 1)
            used[q] += 1

        # pack own dst nodes into blocks by chunk-degree vector
        order_v = np.lexsort((-dq[:, 3], -dq[:, 2], -dq[:, 1], -dq[:, 0],
                              -dq.max(1)))
        slot_of = np.empty(NPC, np.int64)
        slot_of[order_v] = np.arange(NPC)

        per_core_raw.append(dict(u=u, v=v, deg=deg, dq=dq, slot_of=slot_of,
                                 node_chunk=node_chunk, cap=cap))
        dq_all.append(dq)

    # ---------------- shared budgets R[b, q] = max over cores ----------
    R = np.zeros((NBLK, 4), np.int64)
    for c in range(NC):
        dq = dq_all[c]
        slot_of = per_core_raw[c]["slot_of"]
        dqb = np.zeros((NBLK * BLK, 4), np.int64)
        dqb[slot_of] = dq
        R = np.maximum(R, dqb.reshape(NBLK, BLK, 4).max(axis=1))
    # every block needs >= 1 tile so its PSUM region gets written
    empty = R.sum(axis=1) == 0
    R[empty, 0] = 1

    # superbatches: consecutive blocks while tile budget fits T_SB
    blk_tiles = R.sum(axis=1)
    sbs = []
    cur = []
    cur_t = 0
    for b in range(NBLK):
        t = int(blk_tiles[b])
        if cur and cur_t + t > T_SB:
            sbs.append(cur)
            cur, cur_t = [], 0
        cur.append(b)
        cur_t += t
    if cur:
        sbs.append(cur)

    # tile layout: for sb: for q: for b in sb: R[b, q] tiles
    tile_of = {}  # (b, q) -> (tile_start)
    sb_meta = []  # per sb: (t0, tiles, [(q, b, t0_rel, R_bq)...], [(q, t0_rel, ntiles)])
    t_acc = 0
    for blist in sbs:
        runs = []
        qspans = []
        t_rel = 0
        for q in range(4):
            q0 = t_rel
            for b in blist:
                r = int(R[b, q])
                if r == 0:
                    continue
                tile_of[(b, q)] = t_acc + t_rel
                runs.append((q, b, t_rel, r))
                t_rel += r
            if t_rel > q0:
                qspans.append((q, q0, t_rel - q0))
        sb_meta.append(dict(t0=t_acc, tiles=t_rel, blocks=list(blist),
                            runs=runs, qspans=qspans))
        t_acc += t_rel
    TT = t_acc
    POS = TT * 128

    geom = dict(R=R, sbs=sbs, sb_meta=sb_meta, TT=TT, POS=POS)

    # ---------------- per-core edge placement ----------------
    per_core = []
    for c in range(NC):
        pc = per_core_raw[c]
        u, v, deg = pc["u"], pc["v"], pc["deg"]
        slot_of, node_chunk = pc["slot_of"], pc["node_chunk"]

        # column assignment: every node by its chunk over all columns
        col_of = np.full(N, -1, np.int64)
        cols = np.arange(NROWS)
        colchunk = (cols % 128) // CH_PARTS
        zero_col = np.empty(4, np.int64)
        free_cols = []
        for q in range(4):
            qc = cols[colchunk == q]
            zero_col[q] = qc[-1]  # reserved zero column (no node)
            free_cols.append(qc[:-1])
        allnodes = np.arange(N)
        nq = node_chunk[allnodes]
        ptr = [0, 0, 0, 0]
        for q in range(4):
            sel = allnodes[nq == q]
            fc = free_cols[q]
            col_of[sel] = fc[: len(sel)]
            ptr[q] = len(sel)
        rest = allnodes[nq == -1]
        ri = 0
        for q in range(4):
            fc = free_cols[q]
            room = len(fc) - ptr[q]
            take = min(room, len(rest) - ri)
            if take > 0:
                col_of[rest[ri:ri + take]] = fc[ptr[q]:ptr[q] + take]
                ri += take
        assert ri == len(rest), "column capacity exhausted"

        # window-local gather index (maps a column id to its in-window row)
        def col_local(g):
            pseudo = (g % 128) * NT_G + g // 128
            return pseudo - ((g % 128) // CH_PARTS) * CHUNK_PSEUDO

        local = col_local(col_of)

        # edge ranks within (v, q)
        qe = node_chunk[u]
        key = v * 4 + qe
        o = np.argsort(key, kind="stable")
        ks = key[o]
        b0 = np.flatnonzero(np.r_[True, ks[1:] != ks[:-1]])
        cnt2 = np.diff(np.r_[b0, len(ks)])
        rank = np.arange(len(ks)) - np.repeat(b0, cnt2)
        # position per edge
        bfull = slot_of[v[o]]
        blkid = bfull // BLK
        slot = bfull % BLK
        tbase = np.array([tile_of[(int(bb), int(qq))]
                          for bb, qq in zip(blkid, ks % 4)], np.int64)
        posn = (tbase + rank) * 128 + slot

        gs = np.empty(POS, np.int32)
        # pads: per tile the chunk is known; fill with zero col of that chunk
        padfill = np.empty(TT, np.int32)
        for sbm in sb_meta:
            for (q, b, t_rel, r) in sbm["runs"]:
                padfill[sbm["t0"] + t_rel: sbm["t0"] + t_rel + r] = \
                    col_local(zero_col[q])
        gs[:] = np.repeat(padfill, 128)
        gs[posn] = local[u[o]].astype(np.int32)

        # wrap per (sb, q) span
        blocks_w = []
        for sbm in sb_meta:
            t0 = sbm["t0"]
            for (q, q0, ntiles) in sbm["qspans"]:
                lo = (t0 + q0) * 128
                hi = lo + ntiles * 128
                blocks_w.append(_wrap_idx(gs[lo:hi], hi - lo))
        gsrc_idx = np.concatenate(blocks_w, axis=1)

        # pad counts per (slot s, block b): sum_q R[b, q] - deg(node(b, s))
        degfull = np.zeros(NBLK * BLK, np.int64)
        degfull[slot_of] = deg
        padcnt = (R.sum(axis=1)[None, :] -
                  degfull.reshape(NBLK, BLK).T).astype(np.float32)

        per_core.append(dict(gsrc_idx=gsrc_idx, padcnt=padcnt,
                             col_of=col_of, slot_of=slot_of))
    return geom, per_core


def _build(h, W, src, dst):
    h = np.asarray(h, np.float32)
    W = np.asarray(W, np.float32)
    src = np.asarray(src).astype(np.int64)
    dst = np.asarray(dst).astype(np.int64)

    # softmax shift: C = max(0, e_max - 40); bf16 vals hold ex <= e^40.
    z_host = h @ W.T
    e_max = 0.0
    for lo in range(0, len(src), 200000):
        sl = slice(lo, lo + 200000)
        e_max = max(e_max, float(
            np.einsum("ij,ij->i", z_host[src[sl]], z_host[dst[sl]]).max()))
    EXP_SHIFT = max(0.0, e_max - 40.0)

    geom, per_core = _plan(src, dst)
    sb_meta, TT, POS = geom["sb_meta"], geom["TT"], geom["POS"]
    NSB = len(sb_meta)
    T_MAX = max(s["tiles"] for s in sb_meta)

    # ---- host tensors ---------------------------------------------------
    hT = h.T  # [128, N]
    wT = np.ascontiguousarray(W.T).astype(np.float16)
    import ml_dtypes
    ident = np.eye(128, dtype=ml_dtypes.bfloat16)

    in_maps = []
    for c in range(NC):
        hp = np.zeros((IN_DIM, NROWS), np.float16)
        col = per_core[c]["col_of"]
        hp[:, col] = hT.astype(np.float16)
        ho = np.zeros((IN_DIM, NBLK * BLK), np.float16)
        ho[:, per_core[c]["slot_of"]] = hT[
            :, c * NPC:(c + 1) * NPC].astype(np.float16)
        im = dict(hT=hp, hOwn=ho, wT=wT, ident=ident,
                  gsrc_idx=per_core[c]["gsrc_idx"],
                  padcnt=per_core[c]["padcnt"])
        in_maps.append(im)

    NQ = 4
    NSB_RUN = int(os.environ.get("GAT_NSB", NSB))

    def _emit(queue_plan):
        gathers = []

        def _q():
            i = len(gathers)
            if queue_plan is not None and i < len(queue_plan):
                return int(queue_plan[i])
            return 0

        nc = bacc.Bacc(None, target_bir_lowering=False, debug=False,
                       num_swdge_queues=NQ)
        hT_d = nc.declare_dram_parameter("hT", [IN_DIM, NROWS], FP16, isOutput=False)
        hOwn_d = nc.declare_dram_parameter("hOwn", [IN_DIM, NBLK * BLK], FP16, isOutput=False)
        wT_d = nc.declare_dram_parameter("wT", [IN_DIM, OUT_DIM], FP16, isOutput=False)
        ident_d = nc.declare_dram_parameter("ident", [128, 128], BF16, isOutput=False)
        gsrc_d = nc.declare_dram_parameter("gsrc_idx", list(in_maps[0]["gsrc_idx"].shape), I16, isOutput=False)
        padcnt_d = nc.declare_dram_parameter("padcnt", [128, NBLK], F32, isOutput=False)
        out_d = nc.declare_dram_parameter("out", [128 * NBLK, OUT_DIM], F32, isOutput=True)

        z_all = nc.dram_tensor("z_all", [128 * NT_G, 128], FP16)

        with tile.TileContext(nc) as tc:
            with tc.tile_pool(name="cst", bufs=1) as cpool:
                ident_t = cpool.tile([128, 128], BF16)
                nc.sync.dma_start(ident_t[:], ident_d[:])
                padcnt_t = cpool.tile([128, NBLK], F32)
                nc.sync.dma_start(padcnt_t[:], padcnt_d[:])
                z_own = cpool.tile([128, NBLK, OUT_DIM], FP16)
                agg = cpool.tile([128, NBLK, VD], F32)
                shiftt = cpool.tile([128, 1], F32)
                nc.vector.memset(shiftt[:], -EXP_SHIFT)

                # ---------- phase A: z = h @ W^T -------------------------
                with tc.tile_pool(name="w", bufs=1) as wpool, \
                     tc.tile_pool(name="hst", bufs=3) as hpool, \
                     tc.tile_pool(name="psA", bufs=4, space="PSUM") as pspool, \
                     tc.tile_pool(name="zst", bufs=3) as zpool:
                    wt = wpool.tile([IN_DIM, OUT_DIM], FP16)
                    nc.sync.dma_start(wt[:], wT_d[:])
                    z_all3 = z_all[:].rearrange("(p i) d -> p i d", p=128)
                    for i0 in range(0, NT_G, QB):
                        qb = min(QB, NT_G - i0)
                        hstage = hpool.tile([IN_DIM, QB * 128], FP16, tag="hstage")
                        nc.sync.dma_start(hstage[:, : qb * 128],
                                          hT_d[:, i0 * 128:(i0 + qb) * 128])
                        ps = pspool.tile([128, QB, OUT_DIM], F32)
                        for j in range(qb):
                            nc.tensor.matmul(ps[:, j, :],
                                             hstage[:, j * 128:(j + 1) * 128],
                                             wt[:], start=(j == 0),
                                             stop=(j == qb - 1))
                        zstage = zpool.tile([128, QB, 128], FP16,
                                            tag="zstage")
                        nc.vector.memset(zstage[:, :, OUT_DIM:128], 0.0)
                        nc.scalar.activation(zstage[:, :qb, 0:OUT_DIM],
                                             ps[:, :qb, :], AF.Copy)
                        nc.sync.dma_start(z_all3[:, i0:i0 + qb, :],
                                          zstage[:, :qb, :])
                    # z_own: small slot-ordered GEMM (own nodes only)
                    for i0 in range(0, NBLK, QB):
                        qb = min(QB, NBLK - i0)
                        hstage = hpool.tile([IN_DIM, QB * 128], FP16, tag="hstage")
                        nc.sync.dma_start(hstage[:, : qb * 128],
                                          hOwn_d[:, i0 * 128:(i0 + qb) * 128])
                        ps = pspool.tile([128, QB, OUT_DIM], F32)
                        for j in range(qb):
                            nc.tensor.matmul(ps[:, j, :],
                                             hstage[:, j * 128:(j + 1) * 128],
                                             wt[:], start=(j == 0),
                                             stop=(j == qb - 1))
                        nc.scalar.activation(z_own[:, i0:i0 + qb, :],
                                             ps[:, :qb, :], AF.Copy)

                # ---------- phase B: edge superbatches -------------------
                with tc.tile_pool(name="gat", bufs=2) as gpool, \
                     tc.tile_pool(name="pv", bufs=2) as pvpool, \
                     tc.tile_pool(name="sm", bufs=2) as smpool, \
                     tc.tile_pool(name="psB", bufs=8, space="PSUM") as psB, \
                     tc.tile_pool(name="ix", bufs=2) as xpool:

                    def issue(si):
                        sbm = sb_meta[si]
                        t0, Ts = sbm["t0"], sbm["tiles"]
                        zsrc = gpool.tile([128, T_MAX, 128], FP16, tag="zsrc")
                        igs = xpool.tile([128, T_MAX * 8], I16, tag="igs")
                        nc.sync.dma_start(igs[:, : Ts * 8],
                                          gsrc_d[:, t0 * 8: (t0 + Ts) * 8])
                        for (q, q0, ntiles) in sbm["qspans"]:
                            n = ntiles * 128
                            for o2 in range(0, n, GMAX):
                                n2 = min(GMAX, n - o2)
                                g = nc.gpsimd.dma_gather(
                                    zsrc[:, q0 + o2 // 128: q0 + (o2 + n2) // 128, :],
                                    z_all[q * CHUNK_PSEUDO:(q + 1) * CHUNK_PSEUDO, :],
                                    igs[:, q0 * 8 + o2 // 16: q0 * 8 + (o2 + n2) // 16],
                                    n2, n2, 128, single_packet=False,
                                    queue_num=_q())
                                gathers.append(g.ins)
                        return zsrc

                    def compute(si, zsrc):
                        sbm = sb_meta[si]
                        Ts = sbm["tiles"]
                        prod = pvpool.tile([128, T_MAX, OUT_DIM], FP16,
                                           tag="prod")
                        vals = pvpool.tile([128, T_MAX, VD], BF16, tag="vals")
                        for (q, b, t_rel, r) in sbm["runs"]:
                            nc.vector.tensor_mul(
                                prod[:, t_rel:t_rel + r, :],
                                zsrc[:, t_rel:t_rel + r, 0:OUT_DIM],
                                z_own[:, b, None, :].broadcast_to(
                                    (128, r, OUT_DIM)))
                        # halving adds then reduce (fp16 partials of <=4
                        # products stay accurate)
                        nc.vector.tensor_add(prod[:, :Ts, 0:32],
                                             prod[:, :Ts, 0:32],
                                             prod[:, :Ts, 32:64])
                        nc.vector.tensor_add(prod[:, :Ts, 0:16],
                                             prod[:, :Ts, 0:16],
                                             prod[:, :Ts, 16:32])
                        e = smpool.tile([128, T_MAX], F32, tag="e")
                        nc.vector.tensor_reduce(e[:, :Ts], prod[:, :Ts, 0:16],
                                                axis=mybir.AxisListType.X,
                                                op=ALU.add)
                        x1 = smpool.tile([128, T_MAX], F32, tag="x1")
                        nc.scalar.activation(x1[:, :Ts], e[:, :Ts], AF.Exp,
                                             bias=shiftt[:])
                        x2 = smpool.tile([128, T_MAX], F32, tag="x2")
                        nc.scalar.activation(x2[:, :Ts], e[:, :Ts], AF.Exp,
                                             scale=0.2, bias=shiftt[:])
                        ex = smpool.tile([128, T_MAX], F32, tag="ex")
                        nc.vector.tensor_max(ex[:, :Ts], x1[:, :Ts], x2[:, :Ts])
                        nc.vector.tensor_copy(vals[:, :Ts, OUT_DIM],
                                              ex[:, :Ts])
                        # exB broadcast into vals[..0:64], then in-place
                        # vals = zsrc * exB
                        nc.scalar.activation(
                            vals[:, :Ts, 0:OUT_DIM],
                            ex[:, :Ts, None].broadcast_to((128, Ts, OUT_DIM)),
                            AF.Copy)
                        nc.vector.tensor_mul(vals[:, :Ts, 0:OUT_DIM],
                                             zsrc[:, :Ts, 0:OUT_DIM],
                                             vals[:, :Ts, 0:OUT_DIM])
                        # per-block PSUM accumulate + flush
                        tiles_b = {}
                        for (q, b, t_rel, r) in sbm["runs"]:
                            tiles_b.setdefault(b, []).extend(
                                range(t_rel, t_rel + r))
                        for b, tl in tiles_b.items():
                            psb = psB.tile([128, 512], F32, tag="psb")
                            for i, t in enumerate(tl):
                                nc.tensor.matmul(
                                    psb[:, 0:VD], ident_t[:],
                                    vals[:, t, 0:VD],
                                    start=(i == 0), stop=(i == len(tl) - 1))
                            nc.scalar.activation(agg[:, b, :], psb[:, 0:VD],
                                                 AF.Copy)

                    pending = {}
                    for si in range(NSB_RUN + 1):
                        if si < NSB_RUN:
                            pending[si] = issue(si)
                        if si >= 1:
                            compute(si - 1, pending.pop(si - 1))

                # ---------- phase C: pad fix + normalize + elu -----------
                if NSB_RUN == NSB:
                    with tc.tile_pool(name="fin", bufs=1) as fpool:
                        pexn0 = fpool.tile([128, 1], F32)
                        nc.scalar.activation(pexn0[:], shiftt[:], AF.Exp)
                        pexn16 = fpool.tile([128, 1], BF16)
                        nc.vector.tensor_copy(pexn16[:], pexn0[:])
                        pexn = fpool.tile([128, 1], F32)
                        nc.vector.tensor_copy(pexn[:], pexn16[:])
                        padsub = fpool.tile([128, NBLK], F32)
                        nc.vector.tensor_scalar(padsub[:], padcnt_t[:],
                                                pexn[:], None, op0=ALU.mult)
                        d1 = fpool.tile([128, NBLK], F32)
                        nc.vector.tensor_sub(d1[:], agg[:, :, OUT_DIM],
                                             padsub[:])
                        d2 = fpool.tile([128, NBLK], F32)
                        nc.vector.tensor_scalar_add(d2[:], d1[:], 1e-30)
                        rcp = fpool.tile([128, NBLK], F32)
                        nc.vector.reciprocal(rcp[:], d2[:])
                        HB = NBLK // 2  # 49
                        for h0 in range(0, NBLK, HB):
                            o64 = fpool.tile([128, HB, OUT_DIM], F32, tag="o64")
                            nc.vector.tensor_mul(
                                o64[:], agg[:, h0:h0 + HB, 0:OUT_DIM],
                                rcp[:, h0:h0 + HB, None].broadcast_to(
                                    (128, HB, OUT_DIM)))
                            mn = fpool.tile([128, HB, OUT_DIM], F32, tag="mn")
                            nc.vector.tensor_scalar_min(mn[:], o64[:], 0.0)
                            emn = fpool.tile([128, HB, OUT_DIM], F32, tag="emn")
                            nc.scalar.activation(emn[:], mn[:], AF.Exp)
                            mx = fpool.tile([128, HB, OUT_DIM], F32, tag="mx")
                            nc.vector.tensor_scalar_max(mx[:], o64[:], 0.0)
                            res = fpool.tile([128, HB, OUT_DIM], F32, tag="res")
                            nc.vector.scalar_tensor_tensor(
                                res[:], in0=emn[:], scalar=-1.0, in1=mx[:],
                                op0=ALU.add, op1=ALU.add)
                            out_v = out_d[:].rearrange(
                                "(p b) c -> p b c", p=128)[:, h0:h0 + HB, :]
                            nc.sync.dma_start(out_v, res[:])

        nc.finalize()
        return nc, gathers

    from concourse.tile_sem_assignment import PROC_NAME_TO_IDX
    idx_to_lane = {PROC_NAME_TO_IDX[f"DMASW{i}"]: i for i in range(8)}

    def _lanes(gathers):
        out = []
        for g in gathers:
            proc = getattr(g, "bass_scheduled_proc", None)
            out.append(idx_to_lane.get(proc, -1))
        return out

    nc1, g1 = _emit(None)
    lanes = _lanes(g1)
    meta = dict(NC=NC, NPC=NPC, per_core=per_core)
    if all(l >= 0 for l in lanes):
        plan = [l % NQ for l in lanes]
        nc2, g2 = _emit(plan)
        lanes2 = _lanes(g2)
        if all(l >= 0 and l % NQ == q for l, q in zip(lanes2, plan)):
            return nc2, in_maps, meta
    return nc1, in_maps, meta


def kernel(h, W, src, dst):
    global LAST_RESULTS, LAST_BUILD
    nc, in_maps, meta = _build(h, W, src, dst)
    LAST_BUILD = (nc, in_maps, meta)
    results = run_bass_kernel_spmd(
        nc, in_maps, core_ids=list(range(meta["NC"])),
        trace=bool(int(os.environ.get("GAT_TRACE", "0"))),
    )
    LAST_RESULTS = results
    out = np.empty((N, OUT_DIM), np.float32)
    for c in range(meta["NC"]):
        arr = results.results[c]["out"].reshape(128, NBLK, OUT_DIM)
        slot_of = meta["per_core"][c]["slot_of"]
        b = slot_of // BLK
        s = slot_of % BLK
        out[c * NPC:(c + 1) * NPC] = arr[s, b, :]
    return out.astype(np.float32)


# revision 4
# speedup vs baseline: 1.7048x; 1.0408x over previous
"""GAT layer on 8 Trainium2 NeuronCores — identity-slot scheme.

Per core c (SPMD, per-core input maps, shared geometry):
  - Core c owns dst nodes [c*NPC, (c+1)*NPC). Own nodes are assigned to
    (block, slot) positions by a host-side packing that groups nodes with
    similar per-chunk in-degree vectors; the assignment is folded into the
    per-core hT column permutation so phase A produces z_own (SBUF,
    [slot, block, 64] fp16) directly — no on-chip permutation.
  - Sources are assigned to one of 4 gather-window chunks by a greedy
    per-dst balance (keeps per-(block,chunk) max slot-multiplicity ~deg/4),
    folded into the same hT column permutation.
  - Edge stream: per superbatch of blocks, tiles ordered (chunk, block,
    round). A tile holds at most one edge per slot, AT its slot position
    (identity layout) — so no one-hot build, no dst gather, no slot upload:
      prod = zsrc * z_own[block]  (broadcast), e = rowsum, ex = leaky-exp,
      vals = zsrc * ex, psum[block] += vals via PE matmul with identity A.
  - Pads gather a zero z row => vals 0; their exp(-C) denominator
    contribution is subtracted at the end using the device's own exp(-C)
    and a host pad-count tensor.
  - Softmax shift C = e_max - 10 keeps ex <= e^10 in fp16 range; terms
    with alpha < ~e-12 underflow fp16 to 0 (negligible).
"""

import os
import sys

sys.path.insert(0, "/opt/trn_rl_repo")

import numpy as np

import concourse.bacc as bacc
import concourse.mybir as mybir
import concourse.tile as tile
from concourse.bass_utils import run_bass_kernel_spmd

F32 = mybir.dt.float32
BF16 = mybir.dt.bfloat16
FP16 = mybir.dt.float16
I16 = mybir.dt.int16
AF = mybir.ActivationFunctionType
ALU = mybir.AluOpType

LAST_RESULTS = None
LAST_BUILD = None

N = 100000
E_TOT = 1600000
IN_DIM = 128
OUT_DIM = 64
NC = 8
NPC = N // NC  # 12500
NT_G = 680  # GEMM row tiles (covers max per-core source-node count)
NROWS = NT_G * 128  # 87040
N_CHUNK = 4
CH_PARTS = 32
CHUNK_PSEUDO = CH_PARTS * NT_G  # 21760
BLK = 128
NBLK = (NPC + BLK - 1) // BLK  # 98
VD = OUT_DIM + 1  # 65
T_SB = 128  # tiles per superbatch (target)
GMAX = 8192
QB = 8  # GEMM row tiles per stage


def _wrap_idx(idx, budget):
    """[n] int -> [128, budget//16] int16 wrapped + replicated (q7 layout)."""
    a = np.zeros(budget, np.int16)
    a[: len(idx)] = idx.astype(np.int16)
    w = a.reshape(budget // 16, 16).T.copy()
    return np.tile(w, (8, 1))


def _plan(src, dst):
    """Host planning. Returns (geom, per_core)."""
    # ---------------- per-core: chunk assign + packing ----------------
    per_core_raw = []
    dq_all = []
    for c in range(NC):
        m = (dst // NPC) == c
        u = src[m].astype(np.int64)
        v = (dst[m] - c * NPC).astype(np.int64)
        deg = np.bincount(v, minlength=NPC)

        # greedy chunk assignment of src nodes (per-dst balance)
        o = np.argsort(u, kind="stable")
        us, vs = u[o], v[o]
        uniq, starts = np.unique(us, return_index=True)
        ends = np.r_[starts[1:], len(us)]
        cnt = ends - starts
        proc = np.argsort(-cnt, kind="stable")
        node_outcnt = np.zeros(N, np.int64)
        node_outcnt[uniq] = cnt
        dq = np.zeros((NPC, 4), np.int32)
        node_chunk = np.full(N, -1, np.int32)
        # chunk capacities: all columns, minus 1 reserved zero column each
        cols = np.arange(NROWS)
        colchunk = (cols % 128) // CH_PARTS
        cap = np.bincount(colchunk, minlength=4) - 1
        own_lo, own_hi = c * NPC, (c + 1) * NPC
        used = np.zeros(4, np.int64)
        for i in proc:
            nd = int(uniq[i])
            vv = vs[starts[i]:ends[i]]
            score = dq[vv].sum(axis=0).astype(np.float64)
            score[used >= cap] = np.inf
            q = int(np.argmin(score))
            node_chunk[nd] = q
            np.add.at(dq, (vv, q), 1)
            used[q] += 1

        # pack own dst nodes into blocks by chunk-degree vector
        order_v = np.lexsort((-dq[:, 3], -dq[:, 2], -dq[:, 1], -dq[:, 0],
                              -dq.max(1)))
        slot_of = np.empty(NPC, np.int64)
        slot_of[order_v] = np.arange(NPC)

        per_core_raw.append(dict(u=u, v=v, deg=deg, dq=dq, slot_of=slot_of,
                                 node_chunk=node_chunk, cap=cap))
        dq_all.append(dq)

    # ---------------- shared budgets R[b, q] = max over cores ----------
    R = np.zeros((NBLK, 4), np.int64)
    for c in range(NC):
        dq = dq_all[c]
        slot_of = per_core_raw[c]["slot_of"]
        dqb = np.zeros((NBLK * BLK, 4), np.int64)
        dqb[slot_of] = dq
        R = np.maximum(R, dqb.reshape(NBLK, BLK, 4).max(axis=1))
    # every block needs >= 1 tile so its PSUM region gets written
    empty = R.sum(axis=1) == 0
    R[empty, 0] = 1

    # superbatches: consecutive blocks while tile budget fits T_SB
    blk_tiles = R.sum(axis=1)
    sbs = []
    cur = []
    cur_t = 0
    for b in range(NBLK):
        t = int(blk_tiles[b])
        if cur and cur_t + t > T_SB:
            sbs.append(cur)
            cur, cur_t = [], 0
        cur.append(b)
        cur_t += t
    if cur:
        sbs.append(cur)

    # tile layout: for sb: for q: for b in sb: R[b, q] tiles
    tile_of = {}  # (b, q) -> (tile_start)
    sb_meta = []  # per sb: (t0, tiles, [(q, b, t0_rel, R_bq)...], [(q, t0_rel, ntiles)])
    t_acc = 0
    for blist in sbs:
        runs = []
        qspans = []
        t_rel = 0
        for q in range(4):
            q0 = t_rel
            for b in blist:
                r = int(R[b, q])
                if r == 0:
                    continue
                tile_of[(b, q)] = t_acc + t_rel
                runs.append((q, b, t_rel, r))
                t_rel += r
            if t_rel > q0:
                qspans.append((q, q0, t_rel - q0))
        sb_meta.append(dict(t0=t_acc, tiles=t_rel, blocks=list(blist),
                            runs=runs, qspans=qspans))
        t_acc += t_rel
    TT = t_acc
    POS = TT * 128

    geom = dict(R=R, sbs=sbs, sb_meta=sb_meta, TT=TT, POS=POS)

    # ---------------- per-core edge placement ----------------
    per_core = []
    for c in range(NC):
        pc = per_core_raw[c]
        u, v, deg = pc["u"], pc["v"], pc["deg"]
        slot_of, node_chunk = pc["slot_of"], pc["node_chunk"]

        # column assignment: every node by its chunk over all columns
        col_of = np.full(N, -1, np.int64)
        cols = np.arange(NROWS)
        colchunk = (cols % 128) // CH_PARTS
        zero_col = np.empty(4, np.int64)
        free_cols = []
        for q in range(4):
            qc = cols[colchunk == q]
            zero_col[q] = qc[-1]  # reserved zero column (no node)
            free_cols.append(qc[:-1])
        present = np.flatnonzero(node_chunk >= 0)
        nq = node_chunk[present]
        for q in range(4):
            sel = present[nq == q]
            fc = free_cols[q]
            assert len(sel) <= len(fc), "column capacity exhausted"
            col_of[sel] = fc[: len(sel)]

        # window-local gather index (maps a column id to its in-window row)
        def col_local(g):
            pseudo = (g % 128) * NT_G + g // 128
            return pseudo - ((g % 128) // CH_PARTS) * CHUNK_PSEUDO

        local = col_local(col_of)

        # edge ranks within (v, q)
        qe = node_chunk[u]
        key = v * 4 + qe
        o = np.argsort(key, kind="stable")
        ks = key[o]
        b0 = np.flatnonzero(np.r_[True, ks[1:] != ks[:-1]])
        cnt2 = np.diff(np.r_[b0, len(ks)])
        rank = np.arange(len(ks)) - np.repeat(b0, cnt2)
        # position per edge
        bfull = slot_of[v[o]]
        blkid = bfull // BLK
        slot = bfull % BLK
        tbase = np.array([tile_of[(int(bb), int(qq))]
                          for bb, qq in zip(blkid, ks % 4)], np.int64)
        posn = (tbase + rank) * 128 + slot

        gs = np.empty(POS, np.int32)
        # pads: per tile the chunk is known; fill with zero col of that chunk
        padfill = np.empty(TT, np.int32)
        for sbm in sb_meta:
            for (q, b, t_rel, r) in sbm["runs"]:
                padfill[sbm["t0"] + t_rel: sbm["t0"] + t_rel + r] = \
                    col_local(zero_col[q])
        gs[:] = np.repeat(padfill, 128)
        gs[posn] = local[u[o]].astype(np.int32)

        # wrap per (sb, q) span
        blocks_w = []
        for sbm in sb_meta:
            t0 = sbm["t0"]
            for (q, q0, ntiles) in sbm["qspans"]:
                lo = (t0 + q0) * 128
                hi = lo + ntiles * 128
                blocks_w.append(_wrap_idx(gs[lo:hi], hi - lo))
        gsrc_idx = np.concatenate(blocks_w, axis=1)

        # pad counts per (slot s, block b): sum_q R[b, q] - deg(node(b, s))
        degfull = np.zeros(NBLK * BLK, np.int64)
        degfull[slot_of] = deg
        padcnt = (R.sum(axis=1)[None, :] -
                  degfull.reshape(NBLK, BLK).T).astype(np.float32)

        per_core.append(dict(gsrc_idx=gsrc_idx, padcnt=padcnt,
                             col_of=col_of, slot_of=slot_of))
    return geom, per_core


def _build(h, W, src, dst):
    h = np.asarray(h, np.float32)
    W = np.asarray(W, np.float32)
    src = np.asarray(src).astype(np.int64)
    dst = np.asarray(dst).astype(np.int64)

    # softmax shift: C = max(0, e_max - 40); bf16 vals hold ex <= e^40.
    z_host = h @ W.T
    e_max = 0.0
    for lo in range(0, len(src), 200000):
        sl = slice(lo, lo + 200000)
        e_max = max(e_max, float(
            np.einsum("ij,ij->i", z_host[src[sl]], z_host[dst[sl]]).max()))
    EXP_SHIFT = max(0.0, e_max - 40.0)

    geom, per_core = _plan(src, dst)
    sb_meta, TT, POS = geom["sb_meta"], geom["TT"], geom["POS"]
    NSB = len(sb_meta)
    T_MAX = max(s["tiles"] for s in sb_meta)

    # ---- host tensors ---------------------------------------------------
    hT = h.T  # [128, N]
    wT = np.ascontiguousarray(W.T).astype(np.float16)
    import ml_dtypes
    ident = np.eye(128, dtype=ml_dtypes.bfloat16)

    in_maps = []
    for c in range(NC):
        hp = np.zeros((IN_DIM, NROWS), np.float16)
        col = per_core[c]["col_of"]
        pres = col >= 0
        hp[:, col[pres]] = hT[:, pres].astype(np.float16)
        ho = np.zeros((IN_DIM, NBLK * BLK), np.float16)
        ho[:, per_core[c]["slot_of"]] = hT[
            :, c * NPC:(c + 1) * NPC].astype(np.float16)
        im = dict(hT=hp, hOwn=ho, wT=wT, ident=ident,
                  gsrc_idx=per_core[c]["gsrc_idx"],
                  padcnt=per_core[c]["padcnt"])
        in_maps.append(im)

    NQ = 4
    NSB_RUN = int(os.environ.get("GAT_NSB", NSB))

    def _emit(queue_plan):
        gathers = []

        def _q():
            i = len(gathers)
            if queue_plan is not None and i < len(queue_plan):
                return int(queue_plan[i])
            return 0

        nc = bacc.Bacc(None, target_bir_lowering=False, debug=False,
                       num_swdge_queues=NQ)
        hT_d = nc.declare_dram_parameter("hT", [IN_DIM, NROWS], FP16, isOutput=False)
        hOwn_d = nc.declare_dram_parameter("hOwn", [IN_DIM, NBLK * BLK], FP16, isOutput=False)
        wT_d = nc.declare_dram_parameter("wT", [IN_DIM, OUT_DIM], FP16, isOutput=False)
        ident_d = nc.declare_dram_parameter("ident", [128, 128], BF16, isOutput=False)
        gsrc_d = nc.declare_dram_parameter("gsrc_idx", list(in_maps[0]["gsrc_idx"].shape), I16, isOutput=False)
        padcnt_d = nc.declare_dram_parameter("padcnt", [128, NBLK], F32, isOutput=False)
        out_d = nc.declare_dram_parameter("out", [128 * NBLK, OUT_DIM], F32, isOutput=True)

        z_all = nc.dram_tensor("z_all", [128 * NT_G, 128], FP16)

        with tile.TileContext(nc) as tc:
            with tc.tile_pool(name="cst", bufs=1) as cpool:
                ident_t = cpool.tile([128, 128], BF16)
                nc.sync.dma_start(ident_t[:], ident_d[:])
                padcnt_t = cpool.tile([128, NBLK], F32)
                nc.sync.dma_start(padcnt_t[:], padcnt_d[:])
                z_own = cpool.tile([128, NBLK, OUT_DIM], FP16)
                agg = cpool.tile([128, NBLK, VD], F32)
                shiftt = cpool.tile([128, 1], F32)
                nc.vector.memset(shiftt[:], -EXP_SHIFT)

                # ---------- phase A: z = h @ W^T -------------------------
                with tc.tile_pool(name="w", bufs=1) as wpool, \
                     tc.tile_pool(name="hst", bufs=3) as hpool, \
                     tc.tile_pool(name="psA", bufs=4, space="PSUM") as pspool, \
                     tc.tile_pool(name="zst", bufs=3) as zpool:
                    wt = wpool.tile([IN_DIM, OUT_DIM], FP16)
                    nc.sync.dma_start(wt[:], wT_d[:])
                    z_all3 = z_all[:].rearrange("(p i) d -> p i d", p=128)
                    for i0 in range(0, NT_G, QB):
                        qb = min(QB, NT_G - i0)
                        hstage = hpool.tile([IN_DIM, QB * 128], FP16, tag="hstage")
                        nc.sync.dma_start(hstage[:, : qb * 128],
                                          hT_d[:, i0 * 128:(i0 + qb) * 128])
                        ps = pspool.tile([128, QB, OUT_DIM], F32)
                        for j in range(qb):
                            nc.tensor.matmul(ps[:, j, :],
                                             hstage[:, j * 128:(j + 1) * 128],
                                             wt[:], start=(j == 0),
                                             stop=(j == qb - 1))
                        zstage = zpool.tile([128, QB, 128], FP16,
                                            tag="zstage")
                        nc.vector.memset(zstage[:, :, OUT_DIM:128], 0.0)
                        nc.scalar.activation(zstage[:, :qb, 0:OUT_DIM],
                                             ps[:, :qb, :], AF.Copy)
                        nc.sync.dma_start(z_all3[:, i0:i0 + qb, :],
                                          zstage[:, :qb, :])
                    # z_own: small slot-ordered GEMM (own nodes only)
                    for i0 in range(0, NBLK, QB):
                        qb = min(QB, NBLK - i0)
                        hstage = hpool.tile([IN_DIM, QB * 128], FP16, tag="hstage")
                        nc.sync.dma_start(hstage[:, : qb * 128],
                                          hOwn_d[:, i0 * 128:(i0 + qb) * 128])
                        ps = pspool.tile([128, QB, OUT_DIM], F32)
                        for j in range(qb):
                            nc.tensor.matmul(ps[:, j, :],
                                             hstage[:, j * 128:(j + 1) * 128],
                                             wt[:], start=(j == 0),
                                             stop=(j == qb - 1))
                        nc.scalar.activation(z_own[:, i0:i0 + qb, :],
                                             ps[:, :qb, :], AF.Copy)

                # ---------- phase B: edge superbatches -------------------
                with tc.tile_pool(name="gat", bufs=2) as gpool, \
                     tc.tile_pool(name="pv", bufs=2) as pvpool, \
                     tc.tile_pool(name="sm", bufs=2) as smpool, \
                     tc.tile_pool(name="psB", bufs=8, space="PSUM") as psB, \
                     tc.tile_pool(name="ix", bufs=2) as xpool:

                    def issue(si):
                        sbm = sb_meta[si]
                        t0, Ts = sbm["t0"], sbm["tiles"]
                        zsrc = gpool.tile([128, T_MAX, 128], FP16, tag="zsrc")
                        igs = xpool.tile([128, T_MAX * 8], I16, tag="igs")
                        nc.sync.dma_start(igs[:, : Ts * 8],
                                          gsrc_d[:, t0 * 8: (t0 + Ts) * 8])
                        for (q, q0, ntiles) in sbm["qspans"]:
                            n = ntiles * 128
                            for o2 in range(0, n, GMAX):
                                n2 = min(GMAX, n - o2)
                                g = nc.gpsimd.dma_gather(
                                    zsrc[:, q0 + o2 // 128: q0 + (o2 + n2) // 128, :],
                                    z_all[q * CHUNK_PSEUDO:(q + 1) * CHUNK_PSEUDO, :],
                                    igs[:, q0 * 8 + o2 // 16: q0 * 8 + (o2 + n2) // 16],
                                    n2, n2, 128, single_packet=False,
                                    queue_num=_q())
                                gathers.append(g.ins)
                        return zsrc

                    def compute(si, zsrc):
                        sbm = sb_meta[si]
                        Ts = sbm["tiles"]
                        prod = pvpool.tile([128, T_MAX, OUT_DIM], FP16,
                                           tag="prod")
                        vals = pvpool.tile([128, T_MAX, VD], BF16, tag="vals")
                        for (q, b, t_rel, r) in sbm["runs"]:
                            nc.vector.tensor_mul(
                                prod[:, t_rel:t_rel + r, :],
                                zsrc[:, t_rel:t_rel + r, 0:OUT_DIM],
                                z_own[:, b, None, :].broadcast_to(
                                    (128, r, OUT_DIM)))
                        # halving adds then reduce (fp16 partials of <=4
                        # products stay accurate)
                        nc.vector.tensor_add(prod[:, :Ts, 0:32],
                                             prod[:, :Ts, 0:32],
                                             prod[:, :Ts, 32:64])
                        nc.vector.tensor_add(prod[:, :Ts, 0:16],
                                             prod[:, :Ts, 0:16],
                                             prod[:, :Ts, 16:32])
                        e = smpool.tile([128, T_MAX], F32, tag="e")
                        nc.vector.tensor_reduce(e[:, :Ts], prod[:, :Ts, 0:16],
                                                axis=mybir.AxisListType.X,
                                                op=ALU.add)
                        x1 = smpool.tile([128, T_MAX], F32, tag="x1")
                        nc.scalar.activation(x1[:, :Ts], e[:, :Ts], AF.Exp,
                                             bias=shiftt[:])
                        x2 = smpool.tile([128, T_MAX], F32, tag="x2")
                        nc.scalar.activation(x2[:, :Ts], e[:, :Ts], AF.Exp,
                                             scale=0.2, bias=shiftt[:])
                        ex = smpool.tile([128, T_MAX], F32, tag="ex")
                        nc.vector.tensor_max(ex[:, :Ts], x1[:, :Ts], x2[:, :Ts])
                        nc.vector.tensor_copy(vals[:, :Ts, OUT_DIM],
                                              ex[:, :Ts])
                        # exB broadcast into vals[..0:64], then in-place
                        # vals = zsrc * exB
                        nc.scalar.activation(
                            vals[:, :Ts, 0:OUT_DIM],
                            ex[:, :Ts, None].broadcast_to((128, Ts, OUT_DIM)),
                            AF.Copy)
                        nc.vector.tensor_mul(vals[:, :Ts, 0:OUT_DIM],
                                             zsrc[:, :Ts, 0:OUT_DIM],
                                             vals[:, :Ts, 0:OUT_DIM])
                        # per-block PSUM accumulate + flush
                        tiles_b = {}
                        for (q, b, t_rel, r) in sbm["runs"]:
                            tiles_b.setdefault(b, []).extend(
                                range(t_rel, t_rel + r))
                        for b, tl in tiles_b.items():
                            psb = psB.tile([128, 512], F32, tag="psb")
                            for i, t in enumerate(tl):
                                nc.tensor.matmul(
                                    psb[:, 0:VD], ident_t[:],
                                    vals[:, t, 0:VD],
                                    start=(i == 0), stop=(i == len(tl) - 1))
                            nc.scalar.activation(agg[:, b, :], psb[:, 0:VD],
                                                 AF.Copy)

                    pending = {}
                    for si in range(NSB_RUN + 1):
                        if si < NSB_RUN:
                            pending[si] = issue(si)
                        if si >= 1:
                            compute(si - 1, pending.pop(si - 1))

                # ---------- phase C: pad fix + normalize + elu -----------
                if NSB_RUN == NSB:
                    with tc.tile_pool(name="fin", bufs=1) as fpool:
                        pexn0 = fpool.tile([128, 1], F32)
                        nc.scalar.activation(pexn0[:], shiftt[:], AF.Exp)
                        pexn16 = fpool.tile([128, 1], BF16)
                        nc.vector.tensor_copy(pexn16[:], pexn0[:])
                        pexn = fpool.tile([128, 1], F32)
                        nc.vector.tensor_copy(pexn[:], pexn16[:])
                        padsub = fpool.tile([128, NBLK], F32)
                        nc.vector.tensor_scalar(padsub[:], padcnt_t[:],
                                                pexn[:], None, op0=ALU.mult)
                        d1 = fpool.tile([128, NBLK], F32)
                        nc.vector.tensor_sub(d1[:], agg[:, :, OUT_DIM],
                                             padsub[:])
                        d2 = fpool.tile([128, NBLK], F32)
                        nc.vector.tensor_scalar_add(d2[:], d1[:], 1e-30)
                        rcp = fpool.tile([128, NBLK], F32)
                        nc.vector.reciprocal(rcp[:], d2[:])
                        HB = NBLK // 2  # 49
                        for h0 in range(0, NBLK, HB):
                            o64 = fpool.tile([128, HB, OUT_DIM], F32, tag="o64")
                            nc.vector.tensor_mul(
                                o64[:], agg[:, h0:h0 + HB, 0:OUT_DIM],
                                rcp[:, h0:h0 + HB, None].broadcast_to(
                                    (128, HB, OUT_DIM)))
                            mn = fpool.tile([128, HB, OUT_DIM], F32, tag="mn")
                            nc.vector.tensor_scalar_min(mn[:], o64[:], 0.0)
                            emn = fpool.tile([128, HB, OUT_DIM], F32, tag="emn")
                            nc.scalar.activation(emn[:], mn[:], AF.Exp)
                            mx = fpool.tile([128, HB, OUT_DIM], F32, tag="mx")
                            nc.vector.tensor_scalar_max(mx[:], o64[:], 0.0)
                            res = fpool.tile([128, HB, OUT_DIM], F32, tag="res")
                            nc.vector.scalar_tensor_tensor(
                                res[:], in0=emn[:], scalar=-1.0, in1=mx[:],
                                op0=ALU.add, op1=ALU.add)
                            out_v = out_d[:].rearrange(
                                "(p b) c -> p b c", p=128)[:, h0:h0 + HB, :]
                            nc.sync.dma_start(out_v, res[:])

        nc.finalize()
        return nc, gathers

    from concourse.tile_sem_assignment import PROC_NAME_TO_IDX
    idx_to_lane = {PROC_NAME_TO_IDX[f"DMASW{i}"]: i for i in range(8)}

    def _lanes(gathers):
        out = []
        for g in gathers:
            proc = getattr(g, "bass_scheduled_proc", None)
            out.append(idx_to_lane.get(proc, -1))
        return out

    nc1, g1 = _emit(None)
    lanes = _lanes(g1)
    meta = dict(NC=NC, NPC=NPC, per_core=per_core)
    if all(l >= 0 for l in lanes):
        plan = [l % NQ for l in lanes]
        nc2, g2 = _emit(plan)
        lanes2 = _lanes(g2)
        if all(l >= 0 and l % NQ == q for l, q in zip(lanes2, plan)):
            return nc2, in_maps, meta
    return nc1, in_maps, meta


def kernel(h, W, src, dst):
    global LAST_RESULTS, LAST_BUILD
    nc, in_maps, meta = _build(h, W, src, dst)
    LAST_BUILD = (nc, in_maps, meta)
    results = run_bass_kernel_spmd(
        nc, in_maps, core_ids=list(range(meta["NC"])),
        trace=bool(int(os.environ.get("GAT_TRACE", "0"))),
    )
    LAST_RESULTS = results
    out = np.empty((N, OUT_DIM), np.float32)
    for c in range(meta["NC"]):
        arr = results.results[c]["out"].reshape(128, NBLK, OUT_DIM)
        slot_of = meta["per_core"][c]["slot_of"]
        b = slot_of // BLK
        s = slot_of % BLK
        out[c * NPC:(c + 1) * NPC] = arr[s, b, :]
    return out.astype(np.float32)


# revision 6
# speedup vs baseline: 1.7059x; 1.0007x over previous
"""GAT layer on 8 Trainium2 NeuronCores — identity-slot scheme.

Per core c (SPMD, per-core input maps, shared geometry):
  - Core c owns dst nodes [c*NPC, (c+1)*NPC). A host-side packing groups own
    nodes with similar per-chunk in-degree vectors into 128-slot blocks;
    z_own [slot, block, 64] fp16 comes from a small slot-ordered GEMM (hOwn).
  - Source nodes that appear in this core's edges get hT columns, assigned
    to one of 4 gather-window chunks by a greedy per-dst balance; z = h@W^T
    is written to z_all (rows padded to 128 fp16 = 256B for the gather
    descriptor minimum).
  - Edge stream: per superbatch of blocks, tiles ordered (chunk, block,
    round). A tile holds at most one edge per slot, AT its slot position
    (identity layout) — no one-hot build, no dst-side gather, no slot
    upload:
      prod = zsrc * z_own[block]  (broadcast), e = rowsum (fp16 halving
      adds + reduce), ex = max(exp(e-C), exp(0.2e-C)) [= exp(lrelu(e)-C)],
      vals = [zsrc * ex, ex] bf16, psum[block] += vals via PE matmul with
      an identity stationary operand.
  - Pads gather a reserved zero z row => vals 0; their exp(-C) denominator
    contribution is subtracted at the end using the device's own exp(-C)
    (rounded through bf16 to match the stored vals) and a host pad-count.
  - Softmax shift C = max(0, e_max - 40); bf16 vals hold ex <= e^40.
"""

import os
import sys

sys.path.insert(0, "/opt/trn_rl_repo")

import numpy as np

import concourse.bacc as bacc
import concourse.mybir as mybir
import concourse.tile as tile
from concourse.bass_utils import run_bass_kernel_spmd

F32 = mybir.dt.float32
BF16 = mybir.dt.bfloat16
FP16 = mybir.dt.float16
I16 = mybir.dt.int16
AF = mybir.ActivationFunctionType
ALU = mybir.AluOpType

LAST_RESULTS = None
LAST_BUILD = None

N = 100000
E_TOT = 1600000
IN_DIM = 128
OUT_DIM = 64
NC = 8
NPC = N // NC  # 12500
NT_G = 680  # GEMM row tiles (covers max per-core source-node count)
NROWS = NT_G * 128  # 87040
N_CHUNK = 4
CH_PARTS = 32
CHUNK_PSEUDO = CH_PARTS * NT_G  # 21760
BLK = 128
NBLK = (NPC + BLK - 1) // BLK  # 98
VD = OUT_DIM + 1  # 65
T_SB = 128  # tiles per superbatch (target)
GMAX = 8192
QB = 8  # GEMM row tiles per stage


def _wrap_idx(idx, budget):
    """[n] int -> [128, budget//16] int16 wrapped + replicated (q7 layout)."""
    a = np.zeros(budget, np.int16)
    a[: len(idx)] = idx.astype(np.int16)
    w = a.reshape(budget // 16, 16).T.copy()
    return np.tile(w, (8, 1))


def _plan(src, dst):
    """Host planning. Returns (geom, per_core)."""
    # ---------------- per-core: chunk assign + packing ----------------
    per_core_raw = []
    dq_all = []
    for c in range(NC):
        m = (dst // NPC) == c
        u = src[m].astype(np.int64)
        v = (dst[m] - c * NPC).astype(np.int64)
        deg = np.bincount(v, minlength=NPC)

        # greedy chunk assignment of src nodes (per-dst balance)
        o = np.argsort(u, kind="stable")
        us, vs = u[o], v[o]
        uniq, starts = np.unique(us, return_index=True)
        ends = np.r_[starts[1:], len(us)]
        cnt = ends - starts
        proc = np.argsort(-cnt, kind="stable")
        node_outcnt = np.zeros(N, np.int64)
        node_outcnt[uniq] = cnt
        dq = np.zeros((NPC, 4), np.int32)
        node_chunk = np.full(N, -1, np.int32)
        # chunk capacities: all columns, minus 1 reserved zero column each
        cols = np.arange(NROWS)
        colchunk = (cols % 128) // CH_PARTS
        cap = np.bincount(colchunk, minlength=4) - 1
        own_lo, own_hi = c * NPC, (c + 1) * NPC
        used = np.zeros(4, np.int64)
        for i in proc:
            nd = int(uniq[i])
            vv = vs[starts[i]:ends[i]]
            score = dq[vv].sum(axis=0).astype(np.float64)
            score[used >= cap] = np.inf
            q = int(np.argmin(score))
            node_chunk[nd] = q
            np.add.at(dq, (vv, q), 1)
            used[q] += 1

        # pack own dst nodes into blocks by chunk-degree vector
        order_v = np.lexsort((-dq[:, 3], -dq[:, 2], -dq[:, 1], -dq[:, 0],
                              -dq.max(1)))
        slot_of = np.empty(NPC, np.int64)
        slot_of[order_v] = np.arange(NPC)

        per_core_raw.append(dict(u=u, v=v, deg=deg, dq=dq, slot_of=slot_of,
                                 node_chunk=node_chunk, cap=cap))
        dq_all.append(dq)

    # ---------------- shared budgets R[b, q] = max over cores ----------
    R = np.zeros((NBLK, 4), np.int64)
    for c in range(NC):
        dq = dq_all[c]
        slot_of = per_core_raw[c]["slot_of"]
        dqb = np.zeros((NBLK * BLK, 4), np.int64)
        dqb[slot_of] = dq
        R = np.maximum(R, dqb.reshape(NBLK, BLK, 4).max(axis=1))
    # every block needs >= 1 tile so its PSUM region gets written
    empty = R.sum(axis=1) == 0
    R[empty, 0] = 1

    # superbatches: consecutive blocks while tile budget fits T_SB
    blk_tiles = R.sum(axis=1)
    sbs = []
    cur = []
    cur_t = 0
    for b in range(NBLK):
        t = int(blk_tiles[b])
        if cur and cur_t + t > T_SB:
            sbs.append(cur)
            cur, cur_t = [], 0
        cur.append(b)
        cur_t += t
    if cur:
        sbs.append(cur)

    # tile layout: for sb: for q: for b in sb: R[b, q] tiles
    tile_of = {}  # (b, q) -> (tile_start)
    sb_meta = []  # per sb: (t0, tiles, [(q, b, t0_rel, R_bq)...], [(q, t0_rel, ntiles)])
    t_acc = 0
    for blist in sbs:
        runs = []
        qspans = []
        t_rel = 0
        for q in range(4):
            q0 = t_rel
            for b in blist:
                r = int(R[b, q])
                if r == 0:
                    continue
                tile_of[(b, q)] = t_acc + t_rel
                runs.append((q, b, t_rel, r))
                t_rel += r
            if t_rel > q0:
                qspans.append((q, q0, t_rel - q0))
        sb_meta.append(dict(t0=t_acc, tiles=t_rel, blocks=list(blist),
                            runs=runs, qspans=qspans))
        t_acc += t_rel
    TT = t_acc
    POS = TT * 128

    geom = dict(R=R, sbs=sbs, sb_meta=sb_meta, TT=TT, POS=POS)

    # ---------------- per-core edge placement ----------------
    per_core = []
    for c in range(NC):
        pc = per_core_raw[c]
        u, v, deg = pc["u"], pc["v"], pc["deg"]
        slot_of, node_chunk = pc["slot_of"], pc["node_chunk"]

        # column assignment: every node by its chunk over all columns
        col_of = np.full(N, -1, np.int64)
        cols = np.arange(NROWS)
        colchunk = (cols % 128) // CH_PARTS
        zero_col = np.empty(4, np.int64)
        free_cols = []
        for q in range(4):
            qc = cols[colchunk == q]
            zero_col[q] = qc[-1]  # reserved zero column (no node)
            free_cols.append(qc[:-1])
        present = np.flatnonzero(node_chunk >= 0)
        nq = node_chunk[present]
        for q in range(4):
            sel = present[nq == q]
            fc = free_cols[q]
            assert len(sel) <= len(fc), "column capacity exhausted"
            col_of[sel] = fc[: len(sel)]

        # window-local gather index (maps a column id to its in-window row)
        def col_local(g):
            pseudo = (g % 128) * NT_G + g // 128
            return pseudo - ((g % 128) // CH_PARTS) * CHUNK_PSEUDO

        local = col_local(col_of)

        # edge ranks within (v, q)
        qe = node_chunk[u]
        key = v * 4 + qe
        o = np.argsort(key, kind="stable")
        ks = key[o]
        b0 = np.flatnonzero(np.r_[True, ks[1:] != ks[:-1]])
        cnt2 = np.diff(np.r_[b0, len(ks)])
        rank = np.arange(len(ks)) - np.repeat(b0, cnt2)
        # position per edge
        bfull = slot_of[v[o]]
        blkid = bfull // BLK
        slot = bfull % BLK
        tbase = np.array([tile_of[(int(bb), int(qq))]
                          for bb, qq in zip(blkid, ks % 4)], np.int64)
        posn = (tbase + rank) * 128 + slot

        gs = np.empty(POS, np.int32)
        # pads: per tile the chunk is known; fill with zero col of that chunk
        padfill = np.empty(TT, np.int32)
        for sbm in sb_meta:
            for (q, b, t_rel, r) in sbm["runs"]:
                padfill[sbm["t0"] + t_rel: sbm["t0"] + t_rel + r] = \
                    col_local(zero_col[q])
        gs[:] = np.repeat(padfill, 128)
        gs[posn] = local[u[o]].astype(np.int32)

        # wrap per (sb, q) span
        blocks_w = []
        for sbm in sb_meta:
            t0 = sbm["t0"]
            for (q, q0, ntiles) in sbm["qspans"]:
                lo = (t0 + q0) * 128
                hi = lo + ntiles * 128
                blocks_w.append(_wrap_idx(gs[lo:hi], hi - lo))
        gsrc_idx = np.concatenate(blocks_w, axis=1)

        # pad counts per (slot s, block b): sum_q R[b, q] - deg(node(b, s))
        degfull = np.zeros(NBLK * BLK, np.int64)
        degfull[slot_of] = deg
        padcnt = (R.sum(axis=1)[None, :] -
                  degfull.reshape(NBLK, BLK).T).astype(np.float32)

        per_core.append(dict(gsrc_idx=gsrc_idx, padcnt=padcnt,
                             col_of=col_of, slot_of=slot_of))
    return geom, per_core


def _build(h, W, src, dst):
    h = np.asarray(h, np.float32)
    W = np.asarray(W, np.float32)
    src = np.asarray(src).astype(np.int64)
    dst = np.asarray(dst).astype(np.int64)

    # softmax shift: C = max(0, e_max - 40); bf16 vals hold ex <= e^40.
    z_host = h @ W.T
    e_max = 0.0
    for lo in range(0, len(src), 200000):
        sl = slice(lo, lo + 200000)
        e_max = max(e_max, float(
            np.einsum("ij,ij->i", z_host[src[sl]], z_host[dst[sl]]).max()))
    EXP_SHIFT = max(0.0, e_max - 40.0)

    geom, per_core = _plan(src, dst)
    sb_meta, TT, POS = geom["sb_meta"], geom["TT"], geom["POS"]
    NSB = len(sb_meta)
    T_MAX = max(s["tiles"] for s in sb_meta)

    # ---- host tensors ---------------------------------------------------
    hT = h.T  # [128, N]
    wT = np.ascontiguousarray(W.T).astype(np.float16)
    import ml_dtypes
    ident = np.eye(128, dtype=ml_dtypes.bfloat16)

    in_maps = []
    for c in range(NC):
        hp = np.zeros((IN_DIM, NROWS), np.float16)
        col = per_core[c]["col_of"]
        pres = col >= 0
        hp[:, col[pres]] = hT[:, pres].astype(np.float16)
        ho = np.zeros((IN_DIM, NBLK * BLK), np.float16)
        ho[:, per_core[c]["slot_of"]] = hT[
            :, c * NPC:(c + 1) * NPC].astype(np.float16)
        im = dict(hT=hp, hOwn=ho, wT=wT, ident=ident,
                  gsrc_idx=per_core[c]["gsrc_idx"],
                  padcnt=per_core[c]["padcnt"])
        in_maps.append(im)

    NQ = 4
    NSB_RUN = int(os.environ.get("GAT_NSB", NSB))

    def _emit(queue_plan):
        gathers = []

        def _q():
            i = len(gathers)
            if queue_plan is not None and i < len(queue_plan):
                return int(queue_plan[i])
            return 0

        nc = bacc.Bacc(None, target_bir_lowering=False, debug=False,
                       num_swdge_queues=NQ)
        hT_d = nc.declare_dram_parameter("hT", [IN_DIM, NROWS], FP16, isOutput=False)
        hOwn_d = nc.declare_dram_parameter("hOwn", [IN_DIM, NBLK * BLK], FP16, isOutput=False)
        wT_d = nc.declare_dram_parameter("wT", [IN_DIM, OUT_DIM], FP16, isOutput=False)
        ident_d = nc.declare_dram_parameter("ident", [128, 128], BF16, isOutput=False)
        gsrc_d = nc.declare_dram_parameter("gsrc_idx", list(in_maps[0]["gsrc_idx"].shape), I16, isOutput=False)
        padcnt_d = nc.declare_dram_parameter("padcnt", [128, NBLK], F32, isOutput=False)
        out_d = nc.declare_dram_parameter("out", [128 * NBLK, OUT_DIM], F32, isOutput=True)

        z_all = nc.dram_tensor("z_all", [128 * NT_G, 128], FP16)

        with tile.TileContext(nc) as tc:
            with tc.tile_pool(name="cst", bufs=1) as cpool:
                ident_t = cpool.tile([128, 128], BF16)
                nc.sync.dma_start(ident_t[:], ident_d[:])
                padcnt_t = cpool.tile([128, NBLK], F32)
                nc.sync.dma_start(padcnt_t[:], padcnt_d[:])
                z_own = cpool.tile([128, NBLK, OUT_DIM], FP16)
                agg = cpool.tile([128, NBLK, VD], F32)
                shiftt = cpool.tile([128, 1], F32)
                nc.vector.memset(shiftt[:], -EXP_SHIFT)

                # ---------- phase A: z = h @ W^T -------------------------
                with tc.tile_pool(name="w", bufs=1) as wpool, \
                     tc.tile_pool(name="hst", bufs=3) as hpool, \
                     tc.tile_pool(name="psA", bufs=4, space="PSUM") as pspool, \
                     tc.tile_pool(name="zst", bufs=3) as zpool:
                    wt = wpool.tile([IN_DIM, OUT_DIM], FP16)
                    nc.sync.dma_start(wt[:], wT_d[:])
                    z_all3 = z_all[:].rearrange("(p i) d -> p i d", p=128)
                    QW = 2 * QB  # 16-tile DMA granularity
                    for i0 in range(0, NT_G, QW):
                        qw = min(QW, NT_G - i0)
                        hstage = hpool.tile([IN_DIM, QW * 128], FP16, tag="hstage")
                        nc.sync.dma_start(hstage[:, : qw * 128],
                                          hT_d[:, i0 * 128:(i0 + qw) * 128])
                        zstage = zpool.tile([128, QW, 128], FP16,
                                            tag="zstage")
                        nc.vector.memset(zstage[:, :, OUT_DIM:128], 0.0)
                        for h0 in range(0, qw, QB):
                            qb = min(QB, qw - h0)
                            ps = pspool.tile([128, QB, OUT_DIM], F32)
                            for j in range(qb):
                                nc.tensor.matmul(ps[:, j, :],
                                                 hstage[:, (h0 + j) * 128:
                                                        (h0 + j + 1) * 128],
                                                 wt[:], start=(j == 0),
                                                 stop=(j == qb - 1))
                            nc.scalar.activation(
                                zstage[:, h0:h0 + qb, 0:OUT_DIM],
                                ps[:, :qb, :], AF.Copy)
                        nc.sync.dma_start(z_all3[:, i0:i0 + qw, :],
                                          zstage[:, :qw, :])
                    # z_own: small slot-ordered GEMM (own nodes only)
                    for i0 in range(0, NBLK, QB):
                        qb = min(QB, NBLK - i0)
                        hstage = hpool.tile([IN_DIM, QB * 128], FP16, tag="hstage")
                        nc.sync.dma_start(hstage[:, : qb * 128],
                                          hOwn_d[:, i0 * 128:(i0 + qb) * 128])
                        ps = pspool.tile([128, QB, OUT_DIM], F32)
                        for j in range(qb):
                            nc.tensor.matmul(ps[:, j, :],
                                             hstage[:, j * 128:(j + 1) * 128],
                                             wt[:], start=(j == 0),
                                             stop=(j == qb - 1))
                        nc.scalar.activation(z_own[:, i0:i0 + qb, :],
                                             ps[:, :qb, :], AF.Copy)

                # ---------- phase B: edge superbatches -------------------
                with tc.tile_pool(name="gat", bufs=2) as gpool, \
                     tc.tile_pool(name="pv", bufs=2) as pvpool, \
                     tc.tile_pool(name="sm", bufs=2) as smpool, \
                     tc.tile_pool(name="psB", bufs=8, space="PSUM") as psB, \
                     tc.tile_pool(name="ix", bufs=2) as xpool:

                    def issue(si):
                        sbm = sb_meta[si]
                        t0, Ts = sbm["t0"], sbm["tiles"]
                        zsrc = gpool.tile([128, T_MAX, 128], FP16, tag="zsrc")
                        igs = xpool.tile([128, T_MAX * 8], I16, tag="igs")
                        nc.sync.dma_start(igs[:, : Ts * 8],
                                          gsrc_d[:, t0 * 8: (t0 + Ts) * 8])
                        for (q, q0, ntiles) in sbm["qspans"]:
                            n = ntiles * 128
                            for o2 in range(0, n, GMAX):
                                n2 = min(GMAX, n - o2)
                                g = nc.gpsimd.dma_gather(
                                    zsrc[:, q0 + o2 // 128: q0 + (o2 + n2) // 128, :],
                                    z_all[q * CHUNK_PSEUDO:(q + 1) * CHUNK_PSEUDO, :],
                                    igs[:, q0 * 8 + o2 // 16: q0 * 8 + (o2 + n2) // 16],
                                    n2, n2, 128, single_packet=False,
                                    queue_num=_q())
                                gathers.append(g.ins)
                        return zsrc

                    def compute(si, zsrc):
                        sbm = sb_meta[si]
                        Ts = sbm["tiles"]
                        prod = pvpool.tile([128, T_MAX, OUT_DIM], FP16,
                                           tag="prod")
                        vals = pvpool.tile([128, T_MAX, VD], BF16, tag="vals")
                        for (q, b, t_rel, r) in sbm["runs"]:
                            nc.vector.tensor_mul(
                                prod[:, t_rel:t_rel + r, :],
                                zsrc[:, t_rel:t_rel + r, 0:OUT_DIM],
                                z_own[:, b, None, :].broadcast_to(
                                    (128, r, OUT_DIM)))
                        # halving adds then reduce (fp16 partials of <=4
                        # products stay accurate)
                        nc.vector.tensor_add(prod[:, :Ts, 0:32],
                                             prod[:, :Ts, 0:32],
                                             prod[:, :Ts, 32:64])
                        nc.vector.tensor_add(prod[:, :Ts, 0:16],
                                             prod[:, :Ts, 0:16],
                                             prod[:, :Ts, 16:32])
                        e = smpool.tile([128, T_MAX], F32, tag="e")
                        nc.vector.tensor_reduce(e[:, :Ts], prod[:, :Ts, 0:16],
                                                axis=mybir.AxisListType.X,
                                                op=ALU.add)
                        x1 = smpool.tile([128, T_MAX], F32, tag="x1")
                        nc.scalar.activation(x1[:, :Ts], e[:, :Ts], AF.Exp,
                                             bias=shiftt[:])
                        x2 = smpool.tile([128, T_MAX], F32, tag="x2")
                        nc.scalar.activation(x2[:, :Ts], e[:, :Ts], AF.Exp,
                                             scale=0.2, bias=shiftt[:])
                        ex = smpool.tile([128, T_MAX], F32, tag="ex")
                        nc.vector.tensor_max(ex[:, :Ts], x1[:, :Ts], x2[:, :Ts])
                        nc.vector.tensor_copy(vals[:, :Ts, OUT_DIM],
                                              ex[:, :Ts])
                        # exB broadcast into vals[..0:64], then in-place
                        # vals = zsrc * exB
                        nc.scalar.activation(
                            vals[:, :Ts, 0:OUT_DIM],
                            ex[:, :Ts, None].broadcast_to((128, Ts, OUT_DIM)),
                            AF.Copy)
                        nc.vector.tensor_mul(vals[:, :Ts, 0:OUT_DIM],
                                             zsrc[:, :Ts, 0:OUT_DIM],
                                             vals[:, :Ts, 0:OUT_DIM])
                        # per-block PSUM accumulate + flush
                        tiles_b = {}
                        for (q, b, t_rel, r) in sbm["runs"]:
                            tiles_b.setdefault(b, []).extend(
                                range(t_rel, t_rel + r))
                        for b, tl in tiles_b.items():
                            psb = psB.tile([128, 512], F32, tag="psb")
                            for i, t in enumerate(tl):
                                nc.tensor.matmul(
                                    psb[:, 0:VD], ident_t[:],
                                    vals[:, t, 0:VD],
                                    start=(i == 0), stop=(i == len(tl) - 1))
                            nc.scalar.activation(agg[:, b, :], psb[:, 0:VD],
                                                 AF.Copy)

                    pending = {}
                    for si in range(NSB_RUN + 1):
                        if si < NSB_RUN:
                            pending[si] = issue(si)
                        if si >= 1:
                            compute(si - 1, pending.pop(si - 1))

                # ---------- phase C: pad fix + normalize + elu -----------
                if NSB_RUN == NSB:
                    with tc.tile_pool(name="fin", bufs=1) as fpool:
                        pexn0 = fpool.tile([128, 1], F32)
                        nc.scalar.activation(pexn0[:], shiftt[:], AF.Exp)
                        pexn16 = fpool.tile([128, 1], BF16)
                        nc.vector.tensor_copy(pexn16[:], pexn0[:])
                        pexn = fpool.tile([128, 1], F32)
                        nc.vector.tensor_copy(pexn[:], pexn16[:])
                        padsub = fpool.tile([128, NBLK], F32)
                        nc.vector.tensor_scalar(padsub[:], padcnt_t[:],
                                                pexn[:], None, op0=ALU.mult)
                        d1 = fpool.tile([128, NBLK], F32)
                        nc.vector.tensor_sub(d1[:], agg[:, :, OUT_DIM],
                                             padsub[:])
                        d2 = fpool.tile([128, NBLK], F32)
                        nc.vector.tensor_scalar_add(d2[:], d1[:], 1e-30)
                        rcp = fpool.tile([128, NBLK], F32)
                        nc.vector.reciprocal(rcp[:], d2[:])
                        HB = NBLK // 2  # 49
                        for h0 in range(0, NBLK, HB):
                            o64 = fpool.tile([128, HB, OUT_DIM], F32, tag="o64")
                            nc.vector.tensor_mul(
                                o64[:], agg[:, h0:h0 + HB, 0:OUT_DIM],
                                rcp[:, h0:h0 + HB, None].broadcast_to(
                                    (128, HB, OUT_DIM)))
                            mn = fpool.tile([128, HB, OUT_DIM], F32, tag="mn")
                            nc.vector.tensor_scalar_min(mn[:], o64[:], 0.0)
                            emn = fpool.tile([128, HB, OUT_DIM], F32, tag="emn")
                            nc.scalar.activation(emn[:], mn[:], AF.Exp)
                            mx = fpool.tile([128, HB, OUT_DIM], F32, tag="mx")
                            nc.vector.tensor_scalar_max(mx[:], o64[:], 0.0)
                            res = fpool.tile([128, HB, OUT_DIM], F32, tag="res")
                            nc.vector.scalar_tensor_tensor(
                                res[:], in0=emn[:], scalar=-1.0, in1=mx[:],
                                op0=ALU.add, op1=ALU.add)
                            out_v = out_d[:].rearrange(
                                "(p b) c -> p b c", p=128)[:, h0:h0 + HB, :]
                            nc.sync.dma_start(out_v, res[:])

        nc.finalize()
        return nc, gathers

    from concourse.tile_sem_assignment import PROC_NAME_TO_IDX
    idx_to_lane = {PROC_NAME_TO_IDX[f"DMASW{i}"]: i for i in range(8)}

    def _lanes(gathers):
        out = []
        for g in gathers:
            proc = getattr(g, "bass_scheduled_proc", None)
            out.append(idx_to_lane.get(proc, -1))
        return out

    nc1, g1 = _emit(None)
    lanes = _lanes(g1)
    meta = dict(NC=NC, NPC=NPC, per_core=per_core)
    if all(l >= 0 for l in lanes):
        plan = [l % NQ for l in lanes]
        nc2, g2 = _emit(plan)
        lanes2 = _lanes(g2)
        if all(l >= 0 and l % NQ == q for l, q in zip(lanes2, plan)):
            return nc2, in_maps, meta
    return nc1, in_maps, meta


def kernel(h, W, src, dst):
    global LAST_RESULTS, LAST_BUILD
    nc, in_maps, meta = _build(h, W, src, dst)
    LAST_BUILD = (nc, in_maps, meta)
    results = run_bass_kernel_spmd(
        nc, in_maps, core_ids=list(range(meta["NC"])),
        trace=bool(int(os.environ.get("GAT_TRACE", "0"))),
    )
    LAST_RESULTS = results
    out = np.empty((N, OUT_DIM), np.float32)
    for c in range(meta["NC"]):
        arr = results.results[c]["out"].reshape(128, NBLK, OUT_DIM)
        slot_of = meta["per_core"][c]["slot_of"]
        b = slot_of // BLK
        s = slot_of % BLK
        out[c * NPC:(c + 1) * NPC] = arr[s, b, :]
    return out.astype(np.float32)


# revision 7
# speedup vs baseline: 1.8400x; 1.0786x over previous
"""GAT layer on 8 Trainium2 NeuronCores — identity-slot scheme.

Per core c (SPMD, per-core input maps, shared geometry):
  - Core c owns dst nodes [c*NPC, (c+1)*NPC). A host-side packing groups own
    nodes with similar per-chunk in-degree vectors into 128-slot blocks;
    z_own [slot, block, 64] fp16 comes from a small slot-ordered GEMM (hOwn).
  - Source nodes that appear in this core's edges get hT columns, assigned
    to one of 4 gather-window chunks by a greedy per-dst balance; z = h@W^T
    is written to z_all (rows padded to 128 fp16 = 256B for the gather
    descriptor minimum).
  - Edge stream: per superbatch of blocks, tiles ordered (chunk, block,
    round). A tile holds at most one edge per slot, AT its slot position
    (identity layout) — no one-hot build, no dst-side gather, no slot
    upload:
      prod = zsrc * z_own[block]  (broadcast), e = rowsum (fp16 halving
      adds + reduce), ex = max(exp(e-C), exp(0.2e-C)) [= exp(lrelu(e)-C)],
      vals = [zsrc * ex, ex] bf16, psum[block] += vals via PE matmul with
      an identity stationary operand.
  - Pads gather a reserved zero z row => vals 0; their exp(-C) denominator
    contribution is subtracted at the end using the device's own exp(-C)
    (rounded through bf16 to match the stored vals) and a host pad-count.
  - Softmax shift C = max(0, e_max - 40); bf16 vals hold ex <= e^40.
"""

import os
import sys

sys.path.insert(0, "/opt/trn_rl_repo")

import numpy as np

import concourse.bacc as bacc
import concourse.mybir as mybir
import concourse.tile as tile
from concourse.bass_utils import run_bass_kernel_spmd

F32 = mybir.dt.float32
BF16 = mybir.dt.bfloat16
FP16 = mybir.dt.float16
I16 = mybir.dt.int16
AF = mybir.ActivationFunctionType
ALU = mybir.AluOpType

LAST_RESULTS = None
LAST_BUILD = None

N = 100000
E_TOT = 1600000
IN_DIM = 128
OUT_DIM = 64
NC = 8
NPC = N // NC  # 12500
NT_G = 680  # GEMM row tiles (covers max per-core source-node count)
NROWS = NT_G * 128  # 87040
N_CHUNK = 4
CH_PARTS = 32
NT_H = NT_G // 2  # 340 row-pairs per partition
CHUNK_PSEUDO = 64 * NT_H  # 21760 rows per 64-partition window
BLK = 128
NBLK = (NPC + BLK - 1) // BLK  # 98
VD = OUT_DIM + 1  # 65
T_SB = 128  # tiles per superbatch (target)
GMAX = 8192
QB = 8  # GEMM row tiles per stage


def _wrap_idx(idx, budget):
    """[n] int -> [128, budget//16] int16 wrapped + replicated (q7 layout)."""
    a = np.zeros(budget, np.int16)
    a[: len(idx)] = idx.astype(np.int16)
    w = a.reshape(budget // 16, 16).T.copy()
    return np.tile(w, (8, 1))


def _plan(src, dst):
    """Host planning. Returns (geom, per_core)."""
    # ---------------- per-core: chunk assign + packing ----------------
    per_core_raw = []
    dq_all = []
    for c in range(NC):
        m = (dst // NPC) == c
        u = src[m].astype(np.int64)
        v = (dst[m] - c * NPC).astype(np.int64)
        deg = np.bincount(v, minlength=NPC)

        # greedy chunk assignment of src nodes (per-dst balance)
        o = np.argsort(u, kind="stable")
        us, vs = u[o], v[o]
        uniq, starts = np.unique(us, return_index=True)
        ends = np.r_[starts[1:], len(us)]
        cnt = ends - starts
        proc = np.argsort(-cnt, kind="stable")
        node_outcnt = np.zeros(N, np.int64)
        node_outcnt[uniq] = cnt
        dq = np.zeros((NPC, 4), np.int32)
        node_chunk = np.full(N, -1, np.int32)
        # chunk capacities: all columns, minus 1 reserved zero column each
        cols = np.arange(NROWS)
        colchunk = ((cols % 128) // 64) * 2 + (cols // 128) % 2
        cap = np.bincount(colchunk, minlength=4) - 1
        own_lo, own_hi = c * NPC, (c + 1) * NPC
        used = np.zeros(4, np.int64)
        for i in proc:
            nd = int(uniq[i])
            vv = vs[starts[i]:ends[i]]
            score = dq[vv].sum(axis=0).astype(np.float64)
            score[used >= cap] = np.inf
            q = int(np.argmin(score))
            node_chunk[nd] = q
            np.add.at(dq, (vv, q), 1)
            used[q] += 1

        # pack own dst nodes into blocks by chunk-degree vector
        order_v = np.lexsort((-dq[:, 3], -dq[:, 2], -dq[:, 1], -dq[:, 0],
                              -dq.max(1)))
        slot_of = np.empty(NPC, np.int64)
        slot_of[order_v] = np.arange(NPC)

        per_core_raw.append(dict(u=u, v=v, deg=deg, dq=dq, slot_of=slot_of,
                                 node_chunk=node_chunk, cap=cap))
        dq_all.append(dq)

    # ---------------- shared budgets R[b, q] = max over cores ----------
    R = np.zeros((NBLK, 4), np.int64)
    for c in range(NC):
        dq = dq_all[c]
        slot_of = per_core_raw[c]["slot_of"]
        dqb = np.zeros((NBLK * BLK, 4), np.int64)
        dqb[slot_of] = dq
        R = np.maximum(R, dqb.reshape(NBLK, BLK, 4).max(axis=1))
    # every block needs >= 1 tile so its PSUM region gets written
    empty = R.sum(axis=1) == 0
    R[empty, 0] = 1

    # superbatches: consecutive blocks while tile budget fits T_SB
    blk_tiles = R.sum(axis=1)
    sbs = []
    cur = []
    cur_t = 0
    for b in range(NBLK):
        t = int(blk_tiles[b])
        if cur and cur_t + t > T_SB:
            sbs.append(cur)
            cur, cur_t = [], 0
        cur.append(b)
        cur_t += t
    if cur:
        sbs.append(cur)

    # tile layout: for sb: for q: for b in sb: R[b, q] tiles
    tile_of = {}  # (b, q) -> (tile_start)
    sb_meta = []  # per sb: (t0, tiles, [(q, b, t0_rel, R_bq)...], [(q, t0_rel, ntiles)])
    t_acc = 0
    for blist in sbs:
        runs = []
        qspans = []
        t_rel = 0
        for q in range(4):
            q0 = t_rel
            for b in blist:
                r = int(R[b, q])
                if r == 0:
                    continue
                tile_of[(b, q)] = t_acc + t_rel
                runs.append((q, b, t_rel, r))
                t_rel += r
            if t_rel > q0:
                qspans.append((q, q0, t_rel - q0))
        sb_meta.append(dict(t0=t_acc, tiles=t_rel, blocks=list(blist),
                            runs=runs, qspans=qspans))
        t_acc += t_rel
    TT = t_acc
    POS = TT * 128

    geom = dict(R=R, sbs=sbs, sb_meta=sb_meta, TT=TT, POS=POS)

    # ---------------- per-core edge placement ----------------
    per_core = []
    for c in range(NC):
        pc = per_core_raw[c]
        u, v, deg = pc["u"], pc["v"], pc["deg"]
        slot_of, node_chunk = pc["slot_of"], pc["node_chunk"]

        # column assignment: every node by its chunk over all columns
        col_of = np.full(N, -1, np.int64)
        cols = np.arange(NROWS)
        colchunk = ((cols % 128) // 64) * 2 + (cols // 128) % 2
        zero_col = np.empty(4, np.int64)
        free_cols = []
        for q in range(4):
            qc = cols[colchunk == q]
            zero_col[q] = qc[-1]  # reserved zero column (no node)
            free_cols.append(qc[:-1])
        present = np.flatnonzero(node_chunk >= 0)
        nq = node_chunk[present]
        for q in range(4):
            sel = present[nq == q]
            fc = free_cols[q]
            assert len(sel) <= len(fc), "column capacity exhausted"
            col_of[sel] = fc[: len(sel)]

        # window-local gather index (maps a column id to its in-window row)
        def col_local(g):
            return ((g % 128) % 64) * NT_H + (g // 128) // 2

        local = col_local(col_of)

        # edge ranks within (v, q)
        qe = node_chunk[u]
        key = v * 4 + qe
        o = np.argsort(key, kind="stable")
        ks = key[o]
        b0 = np.flatnonzero(np.r_[True, ks[1:] != ks[:-1]])
        cnt2 = np.diff(np.r_[b0, len(ks)])
        rank = np.arange(len(ks)) - np.repeat(b0, cnt2)
        # position per edge
        bfull = slot_of[v[o]]
        blkid = bfull // BLK
        slot = bfull % BLK
        tbase = np.array([tile_of[(int(bb), int(qq))]
                          for bb, qq in zip(blkid, ks % 4)], np.int64)
        posn = (tbase + rank) * 128 + slot

        gs = np.empty(POS, np.int32)
        # pads: per tile the chunk is known; fill with zero col of that chunk
        padfill = np.empty(TT, np.int32)
        for sbm in sb_meta:
            for (q, b, t_rel, r) in sbm["runs"]:
                padfill[sbm["t0"] + t_rel: sbm["t0"] + t_rel + r] = \
                    col_local(zero_col[q])
        gs[:] = np.repeat(padfill, 128)
        gs[posn] = local[u[o]].astype(np.int32)

        # wrap per (sb, q) span
        blocks_w = []
        for sbm in sb_meta:
            t0 = sbm["t0"]
            for (q, q0, ntiles) in sbm["qspans"]:
                lo = (t0 + q0) * 128
                hi = lo + ntiles * 128
                blocks_w.append(_wrap_idx(gs[lo:hi], hi - lo))
        gsrc_idx = np.concatenate(blocks_w, axis=1)

        # pad counts per (slot s, block b): sum_q R[b, q] - deg(node(b, s))
        degfull = np.zeros(NBLK * BLK, np.int64)
        degfull[slot_of] = deg
        padcnt = (R.sum(axis=1)[None, :] -
                  degfull.reshape(NBLK, BLK).T).astype(np.float32)

        per_core.append(dict(gsrc_idx=gsrc_idx, padcnt=padcnt,
                             col_of=col_of, slot_of=slot_of))
    return geom, per_core


def _build(h, W, src, dst):
    h = np.asarray(h, np.float32)
    W = np.asarray(W, np.float32)
    src = np.asarray(src).astype(np.int64)
    dst = np.asarray(dst).astype(np.int64)

    # softmax shift: C = max(0, e_max - 40); bf16 vals hold ex <= e^40.
    z_host = h @ W.T
    e_max = 0.0
    for lo in range(0, len(src), 200000):
        sl = slice(lo, lo + 200000)
        e_max = max(e_max, float(
            np.einsum("ij,ij->i", z_host[src[sl]], z_host[dst[sl]]).max()))
    EXP_SHIFT = max(0.0, e_max - 40.0)

    geom, per_core = _plan(src, dst)
    sb_meta, TT, POS = geom["sb_meta"], geom["TT"], geom["POS"]
    NSB = len(sb_meta)
    T_MAX = max(s["tiles"] for s in sb_meta)

    # ---- host tensors ---------------------------------------------------
    hT = h.T  # [128, N]
    wT = np.ascontiguousarray(W.T).astype(np.float16)
    import ml_dtypes
    ident = np.eye(128, dtype=ml_dtypes.bfloat16)

    in_maps = []
    for c in range(NC):
        hp = np.zeros((IN_DIM, NROWS), np.float16)
        col = per_core[c]["col_of"]
        pres = col >= 0
        hp[:, col[pres]] = hT[:, pres].astype(np.float16)
        ho = np.zeros((IN_DIM, NBLK * BLK), np.float16)
        ho[:, per_core[c]["slot_of"]] = hT[
            :, c * NPC:(c + 1) * NPC].astype(np.float16)
        im = dict(hT=hp, hOwn=ho, wT=wT, ident=ident,
                  gsrc_idx=per_core[c]["gsrc_idx"],
                  padcnt=per_core[c]["padcnt"])
        in_maps.append(im)

    NQ = 4
    NSB_RUN = int(os.environ.get("GAT_NSB", NSB))

    def _emit(queue_plan):
        gathers = []

        def _q():
            i = len(gathers)
            if queue_plan is not None and i < len(queue_plan):
                return int(queue_plan[i])
            return 0

        nc = bacc.Bacc(None, target_bir_lowering=False, debug=False,
                       num_swdge_queues=NQ)
        hT_d = nc.declare_dram_parameter("hT", [IN_DIM, NROWS], FP16, isOutput=False)
        hOwn_d = nc.declare_dram_parameter("hOwn", [IN_DIM, NBLK * BLK], FP16, isOutput=False)
        wT_d = nc.declare_dram_parameter("wT", [IN_DIM, OUT_DIM], FP16, isOutput=False)
        ident_d = nc.declare_dram_parameter("ident", [128, 128], BF16, isOutput=False)
        gsrc_d = nc.declare_dram_parameter("gsrc_idx", list(in_maps[0]["gsrc_idx"].shape), I16, isOutput=False)
        padcnt_d = nc.declare_dram_parameter("padcnt", [128, NBLK], F32, isOutput=False)
        out_d = nc.declare_dram_parameter("out", [128 * NBLK, OUT_DIM], F32, isOutput=True)

        z_all = nc.dram_tensor("z_all", [128 * NT_H, 128], FP16)

        with tile.TileContext(nc) as tc:
            with tc.tile_pool(name="cst", bufs=1) as cpool:
                ident_t = cpool.tile([128, 128], BF16)
                nc.sync.dma_start(ident_t[:], ident_d[:])
                padcnt_t = cpool.tile([128, NBLK], F32)
                nc.sync.dma_start(padcnt_t[:], padcnt_d[:])
                z_own = cpool.tile([128, NBLK, OUT_DIM], FP16)
                agg = cpool.tile([128, NBLK, VD], F32)
                shiftt = cpool.tile([128, 1], F32)
                nc.vector.memset(shiftt[:], -EXP_SHIFT)

                # ---------- phase A: z = h @ W^T -------------------------
                with tc.tile_pool(name="w", bufs=1) as wpool, \
                     tc.tile_pool(name="hst", bufs=3) as hpool, \
                     tc.tile_pool(name="psA", bufs=4, space="PSUM") as pspool, \
                     tc.tile_pool(name="zst", bufs=3) as zpool:
                    wt = wpool.tile([IN_DIM, OUT_DIM], FP16)
                    nc.sync.dma_start(wt[:], wT_d[:])
                    z_all3 = z_all[:].rearrange("(p i) d -> p i d", p=128)
                    QW = 2 * QB  # 16-tile DMA granularity (even: pairs pack)
                    for i0 in range(0, NT_G, QW):
                        qw = min(QW, NT_G - i0)
                        hstage = hpool.tile([IN_DIM, QW * 128], FP16, tag="hstage")
                        nc.sync.dma_start(hstage[:, : qw * 128],
                                          hT_d[:, i0 * 128:(i0 + qw) * 128])
                        zstage = zpool.tile([128, QW, OUT_DIM], FP16,
                                            tag="zstage")
                        for h0 in range(0, qw, QB):
                            qb = min(QB, qw - h0)
                            ps = pspool.tile([128, QB, OUT_DIM], F32)
                            for j in range(qb):
                                nc.tensor.matmul(ps[:, j, :],
                                                 hstage[:, (h0 + j) * 128:
                                                        (h0 + j + 1) * 128],
                                                 wt[:], start=(j == 0),
                                                 stop=(j == qb - 1))
                            nc.scalar.activation(
                                zstage[:, h0:h0 + qb, :],
                                ps[:, :qb, :], AF.Copy)
                        nc.sync.dma_start(
                            z_all3[:, i0 // 2:(i0 + qw) // 2, :]
                                .rearrange("p a c -> p (a c)"),
                            zstage[:, :qw, :].rearrange("p a c -> p (a c)"))
                    # z_own: small slot-ordered GEMM (own nodes only)
                    for i0 in range(0, NBLK, QB):
                        qb = min(QB, NBLK - i0)
                        hstage = hpool.tile([IN_DIM, QB * 128], FP16, tag="hstage")
                        nc.sync.dma_start(hstage[:, : qb * 128],
                                          hOwn_d[:, i0 * 128:(i0 + qb) * 128])
                        ps = pspool.tile([128, QB, OUT_DIM], F32)
                        for j in range(qb):
                            nc.tensor.matmul(ps[:, j, :],
                                             hstage[:, j * 128:(j + 1) * 128],
                                             wt[:], start=(j == 0),
                                             stop=(j == qb - 1))
                        nc.scalar.activation(z_own[:, i0:i0 + qb, :],
                                             ps[:, :qb, :], AF.Copy)

                # ---------- phase B: edge superbatches -------------------
                with tc.tile_pool(name="gat", bufs=2) as gpool, \
                     tc.tile_pool(name="pv", bufs=2) as pvpool, \
                     tc.tile_pool(name="sm", bufs=2) as smpool, \
                     tc.tile_pool(name="psB", bufs=8, space="PSUM") as psB, \
                     tc.tile_pool(name="ix", bufs=2) as xpool:

                    def issue(si):
                        sbm = sb_meta[si]
                        t0, Ts = sbm["t0"], sbm["tiles"]
                        zsrc = gpool.tile([128, T_MAX, 128], FP16, tag="zsrc")
                        igs = xpool.tile([128, T_MAX * 8], I16, tag="igs")
                        nc.sync.dma_start(igs[:, : Ts * 8],
                                          gsrc_d[:, t0 * 8: (t0 + Ts) * 8])
                        for (q, q0, ntiles) in sbm["qspans"]:
                            n = ntiles * 128
                            for o2 in range(0, n, GMAX):
                                n2 = min(GMAX, n - o2)
                                g = nc.gpsimd.dma_gather(
                                    zsrc[:, q0 + o2 // 128: q0 + (o2 + n2) // 128, :],
                                    z_all[(q // 2) * CHUNK_PSEUDO:
                                          (q // 2 + 1) * CHUNK_PSEUDO, :],
                                    igs[:, q0 * 8 + o2 // 16: q0 * 8 + (o2 + n2) // 16],
                                    n2, n2, 128, single_packet=False,
                                    queue_num=_q())
                                gathers.append(g.ins)
                        return zsrc

                    def compute(si, zsrc):
                        sbm = sb_meta[si]
                        Ts = sbm["tiles"]
                        prod = pvpool.tile([128, T_MAX, OUT_DIM], FP16,
                                           tag="prod")
                        vals = pvpool.tile([128, T_MAX, VD], BF16, tag="vals")
                        for (q, b, t_rel, r) in sbm["runs"]:
                            h0 = (q % 2) * OUT_DIM
                            nc.vector.tensor_mul(
                                prod[:, t_rel:t_rel + r, :],
                                zsrc[:, t_rel:t_rel + r, h0:h0 + OUT_DIM],
                                z_own[:, b, None, :].broadcast_to(
                                    (128, r, OUT_DIM)))
                        # halving adds then reduce (fp16 partials of <=4
                        # products stay accurate)
                        nc.vector.tensor_add(prod[:, :Ts, 0:32],
                                             prod[:, :Ts, 0:32],
                                             prod[:, :Ts, 32:64])
                        nc.vector.tensor_add(prod[:, :Ts, 0:16],
                                             prod[:, :Ts, 0:16],
                                             prod[:, :Ts, 16:32])
                        e = smpool.tile([128, T_MAX], F32, tag="e")
                        nc.vector.tensor_reduce(e[:, :Ts], prod[:, :Ts, 0:16],
                                                axis=mybir.AxisListType.X,
                                                op=ALU.add)
                        x1 = smpool.tile([128, T_MAX], F32, tag="x1")
                        nc.scalar.activation(x1[:, :Ts], e[:, :Ts], AF.Exp,
                                             bias=shiftt[:])
                        x2 = smpool.tile([128, T_MAX], F32, tag="x2")
                        nc.scalar.activation(x2[:, :Ts], e[:, :Ts], AF.Exp,
                                             scale=0.2, bias=shiftt[:])
                        ex = smpool.tile([128, T_MAX], F32, tag="ex")
                        nc.vector.tensor_max(ex[:, :Ts], x1[:, :Ts], x2[:, :Ts])
                        nc.vector.tensor_copy(vals[:, :Ts, OUT_DIM],
                                              ex[:, :Ts])
                        # exB broadcast into vals[..0:64], then in-place
                        # vals = zsrc * exB
                        nc.scalar.activation(
                            vals[:, :Ts, 0:OUT_DIM],
                            ex[:, :Ts, None].broadcast_to((128, Ts, OUT_DIM)),
                            AF.Copy)
                        for (q, q0, ntiles) in sbm["qspans"]:
                            h0 = (q % 2) * OUT_DIM
                            nc.vector.tensor_mul(
                                vals[:, q0:q0 + ntiles, 0:OUT_DIM],
                                zsrc[:, q0:q0 + ntiles, h0:h0 + OUT_DIM],
                                vals[:, q0:q0 + ntiles, 0:OUT_DIM])
                        # per-block PSUM accumulate + flush
                        tiles_b = {}
                        for (q, b, t_rel, r) in sbm["runs"]:
                            tiles_b.setdefault(b, []).extend(
                                range(t_rel, t_rel + r))
                        for b, tl in tiles_b.items():
                            psb = psB.tile([128, 512], F32, tag="psb")
                            for i, t in enumerate(tl):
                                nc.tensor.matmul(
                                    psb[:, 0:VD], ident_t[:],
                                    vals[:, t, 0:VD],
                                    start=(i == 0), stop=(i == len(tl) - 1))
                            nc.scalar.activation(agg[:, b, :], psb[:, 0:VD],
                                                 AF.Copy)

                    pending = {}
                    for si in range(NSB_RUN + 1):
                        if si < NSB_RUN:
                            pending[si] = issue(si)
                        if si >= 1:
                            compute(si - 1, pending.pop(si - 1))

                # ---------- phase C: pad fix + normalize + elu -----------
                if NSB_RUN == NSB:
                    with tc.tile_pool(name="fin", bufs=1) as fpool:
                        pexn0 = fpool.tile([128, 1], F32)
                        nc.scalar.activation(pexn0[:], shiftt[:], AF.Exp)
                        pexn16 = fpool.tile([128, 1], BF16)
                        nc.vector.tensor_copy(pexn16[:], pexn0[:])
                        pexn = fpool.tile([128, 1], F32)
                        nc.vector.tensor_copy(pexn[:], pexn16[:])
                        padsub = fpool.tile([128, NBLK], F32)
                        nc.vector.tensor_scalar(padsub[:], padcnt_t[:],
                                                pexn[:], None, op0=ALU.mult)
                        d1 = fpool.tile([128, NBLK], F32)
                        nc.vector.tensor_sub(d1[:], agg[:, :, OUT_DIM],
                                             padsub[:])
                        d2 = fpool.tile([128, NBLK], F32)
                        nc.vector.tensor_scalar_add(d2[:], d1[:], 1e-30)
                        rcp = fpool.tile([128, NBLK], F32)
                        nc.vector.reciprocal(rcp[:], d2[:])
                        HB = NBLK // 2  # 49
                        for h0 in range(0, NBLK, HB):
                            o64 = fpool.tile([128, HB, OUT_DIM], F32, tag="o64")
                            nc.vector.tensor_mul(
                                o64[:], agg[:, h0:h0 + HB, 0:OUT_DIM],
                                rcp[:, h0:h0 + HB, None].broadcast_to(
                                    (128, HB, OUT_DIM)))
                            mn = fpool.tile([128, HB, OUT_DIM], F32, tag="mn")
                            nc.vector.tensor_scalar_min(mn[:], o64[:], 0.0)
                            emn = fpool.tile([128, HB, OUT_DIM], F32, tag="emn")
                            nc.scalar.activation(emn[:], mn[:], AF.Exp)
                            mx = fpool.tile([128, HB, OUT_DIM], F32, tag="mx")
                            nc.vector.tensor_scalar_max(mx[:], o64[:], 0.0)
                            res = fpool.tile([128, HB, OUT_DIM], F32, tag="res")
                            nc.vector.scalar_tensor_tensor(
                                res[:], in0=emn[:], scalar=-1.0, in1=mx[:],
                                op0=ALU.add, op1=ALU.add)
                            out_v = out_d[:].rearrange(
                                "(p b) c -> p b c", p=128)[:, h0:h0 + HB, :]
                            nc.sync.dma_start(out_v, res[:])

        nc.finalize()
        return nc, gathers

    from concourse.tile_sem_assignment import PROC_NAME_TO_IDX
    idx_to_lane = {PROC_NAME_TO_IDX[f"DMASW{i}"]: i for i in range(8)}

    def _lanes(gathers):
        out = []
        for g in gathers:
            proc = getattr(g, "bass_scheduled_proc", None)
            out.append(idx_to_lane.get(proc, -1))
        return out

    nc1, g1 = _emit(None)
    lanes = _lanes(g1)
    meta = dict(NC=NC, NPC=NPC, per_core=per_core)
    if all(l >= 0 for l in lanes):
        plan = [l % NQ for l in lanes]
        nc2, g2 = _emit(plan)
        lanes2 = _lanes(g2)
        if all(l >= 0 and l % NQ == q for l, q in zip(lanes2, plan)):
            return nc2, in_maps, meta
    return nc1, in_maps, meta


def kernel(h, W, src, dst):
    global LAST_RESULTS, LAST_BUILD
    nc, in_maps, meta = _build(h, W, src, dst)
    LAST_BUILD = (nc, in_maps, meta)
    results = run_bass_kernel_spmd(
        nc, in_maps, core_ids=list(range(meta["NC"])),
        trace=bool(int(os.environ.get("GAT_TRACE", "0"))),
    )
    LAST_RESULTS = results
    out = np.empty((N, OUT_DIM), np.float32)
    for c in range(meta["NC"]):
        arr = results.results[c]["out"].reshape(128, NBLK, OUT_DIM)
        slot_of = meta["per_core"][c]["slot_of"]
        b = slot_of // BLK
        s = slot_of % BLK
        out[c * NPC:(c + 1) * NPC] = arr[s, b, :]
    return out.astype(np.float32)
